# revision 31
# baseline (speedup 1.0000x reference)
"""CNN + truncated path-signature (depth 4) + FF head on 8 TRN2 NeuronCores.

Strategy
--------
- Batch data-parallel signature computation: core c handles batches
  [8c, 8c+8) = 32 (batch, out_ch) lanes, T=128 time steps on partitions.
- Signature reformulated to avoid sequential scans:
    dx, P1 (= shifted path), Y (= suffix sums) are free;
    the only prefix scan (level 2, s2) is one triangular matmul L @ m2;
    S3 = bt^T dx;  S4 = bt^T (dx x Y) + t8^T (dx x dx)/2,
  so levels 3 and 4 are plain T-contractions on the TensorEngine.
- All PE contractions run in fp16 (1 cycle/row; fp32 modes are 4x slower),
  accumulating in fp32 PSUM.  fp16's 10-bit mantissa keeps the final
  rel-err ~2.5e-3 (bf16 operands measured 1.3e-2, too close to the gate).
- w0 (60 MB fp32) is row-sharded 8 ways as fp16 (3.8 MB/core stream):
  AllToAll re-shards the fp16 signature activations feature-wise, each
  core multiplies its [3690, 512] w0 shard for all 64 batches, and a
  fp16 ReduceScatter returns each core its own 8 batches.
- A2A buffer layout is [batch, K] (K fastest) so both the pack DMAs
  (180 B runs) and the post-A2A lhsT gathers (256 B runs) avoid the
  16-byte-run descriptor storm the K-major layout suffers.
- w0 rows are permuted and pre-scaled host-side (S3/S4 features are
  produced /4 on device to keep fp16 headroom), so no on-device
  transposes or rescales are needed.
"""
import os
import sys
sys.path.insert(0, "/opt/trn_rl_repo")
if os.environ.get("JAX_PLATFORMS") == "cpu":
    # The SPMD launch needs the axon/neuron PJRT backend.
    os.environ["JAX_PLATFORMS"] = ""

import numpy as np
import bass_rust as _bass_rust
import concourse.bass as bass
import concourse.tile as tile
import concourse.mybir as mybir
from concourse.vector_clock import ScopedClock
from concourse.bass_utils import run_bass_kernel_spmd

F32 = mybir.dt.float32
F16 = mybir.dt.float16
AL = mybir.AluOpType
AF = mybir.ActivationFunctionType

NCORES = 8
B, T, IN_CH = 64, 128, 32
OUT_CH, CH, D = 4, 8, 9            # conv out-channels, conv width, path dim
BL = B // NCORES                   # local batches = 8
LANES = BL * OUT_CH                # 32 lanes/core
NG = 4                             # lane groups == out-channels
GL = 8                             # lanes per group == local batches
SIGC = 7380                        # per-lane signature channels
KSH = OUT_CH * SIGC // NCORES      # w0 K-shard rows per core = 3690
NCH = 29                           # K chunks of 128
KSHP = NCH * 128                   # shard padded to 3712 (xbar %128 rule)
H0, H1, NCLS = 512, 256, 10
FSCALE = 16.0                      # S3/S4 features arrive /16 (fp16 headroom)
W = LANES * D                      # 288


class _SplitDrainTileContext(tile.TileContext):
    """Tile exit drain carries one sem wait per CTRL instruction.

    This container's walrus build rejects >2 sync waits on a CTRL
    instruction; Tile's exit drain waits on the whole global clock.
    Redistribute the waits over nops on the same engine (program order on
    one engine preserves semantics)."""

    MAX_WAITS = 1

    def _split_body_waits(self):
        """Move excess sem waits from any instruction onto preceding nops on
        the same engine (same-engine program order preserves semantics)."""
        nc = self.nc
        for bb in nc.main_func.blocks:
            heavy = [ins for ins in bb.instructions
                     if ins.sync_info and ins.sync_info.on_wait
                     and len(ins.sync_info.on_wait) > self.MAX_WAITS]
            if not heavy:
                continue
            extra = {}
            for ins in heavy:
                w = list(ins.sync_info.on_wait)
                ins.sync_info.on_wait = w[:self.MAX_WAITS]
                nops = []
                for i in range(self.MAX_WAITS, len(w), self.MAX_WAITS):
                    n = nc.engines[ins.engine].nop(hint="wait_split")
                    # pop the freshly appended nop from wherever it landed
                    for bb2 in nc.main_func.blocks:
                        if bb2.instructions and bb2.instructions[-1] is n.ins:
                            bb2.instructions.pop()
                            break
                    for wt in w[i:i + self.MAX_WAITS]:
                        handle = _bass_rust.SemaphoreHandle(wt.ant_name, wt.id)
                        _bass_rust.wait_op(n.ins, handle, wt.wait_value,
                                           "sem-ge", False)
                    nops.append(n.ins)
                extra[id(ins)] = nops
            new_list = []
            for ins in bb.instructions:
                new_list.extend(extra.get(id(ins), ()))
                new_list.append(ins)
            bb.instructions[:] = new_list

    def _drain_and_barrier(self, tick_clock, wait_clock):
        nc = self.nc
        self._split_body_waits()
        probe = nc.sync.nop(hint="tile_exit_wait_0")
        wait_clock.add_sem_waits(
            probe.ins, ScopedClock({None: tick_clock.global_clock})
        )
        waits = list(probe.ins.sync_info.on_wait or [])
        probe.ins.sync_info.on_wait = waits[:1]
        for w in waits[1:]:
            n = nc.sync.nop(hint="tile_exit_wait")
            handle = _bass_rust.SemaphoreHandle(w.ant_name, w.id)
            _bass_rust.wait_op(n.ins, handle, w.wait_value, "sem-ge", False)
        nc.sync.drain()
        nc.all_engine_barrier()
        assert self.sems is not None
        popped = nc._tile_sem_poison_stack.pop()
        assert popped is self._sem_poison
        nc.clear_and_free_semaphores(list(self.sems.allocated().values()))
        nc.all_engine_barrier()


def _ap(t, extra, *dims):
    """AP over tile t's buffer: partition dim from the tile, custom free dims.

    dims[0] may override the partition [step, count]."""
    base = t[:]
    return bass.AP(base.tensor, base.offset + extra, list(dims))


def _w0_perm():
    """Row permutation p s.t. w0_permuted[i] = w0[p[i]] matches the kernel's
    feature order: per oc: [S1(9) | S2(81)] then rows 90 + ij*90 + c with
    c<81 -> level4 (ij,kl=c), c>=81 -> level3 (ij, k=c-81)."""
    p = np.empty(OUT_CH * SIGC, np.int64)
    i = 0
    for oc in range(OUT_CH):
        base = oc * SIGC
        p[i:i + 90] = base + np.arange(90)          # S1 then S2, native order
        i += 90
        for ij in range(81):
            # level-4 block (ij, kl) for kl in 0..80 -> orig 819 + ij*81 + kl
            p[i:i + 81] = base + 819 + ij * 81 + np.arange(81)
            i += 81
            # level-3 (ij, k) for k in 0..8 -> orig 90 + ij*9 + k
            p[i:i + 9] = base + 90 + ij * 9 + np.arange(9)
            i += 9
    assert i == OUT_CH * SIGC
    return p


def _build():
    nc = bass.Bass(num_devices=NCORES, target_bir_lowering=True, trn_type="TRN2")

    # ---- per-core DRAM inputs ----
    xs = nc.dram_tensor("xs", [T, BL, IN_CH], F32, kind="ExternalInput")
    cwr = nc.dram_tensor("cwr", [128, 16], F32, kind="ExternalInput")
    cbr = nc.dram_tensor("cbr", [128, OUT_CH], F32, kind="ExternalInput")
    tlin = nc.dram_tensor("tlin", [128, 1], F32, kind="ExternalInput")
    ltri = nc.dram_tensor("ltri", [128, 128], F16, kind="ExternalInput")
    onec = nc.dram_tensor("onec", [128, 1], F16, kind="ExternalInput")
    idh = nc.dram_tensor("idh", [64, 64], F16, kind="ExternalInput")
    onef = nc.dram_tensor("onef", [128, 1], F32, kind="ExternalInput")
    idn = nc.dram_tensor("idn", [128, BL], F32, kind="ExternalInput")
    w0s = nc.dram_tensor("w0s", [KSHP, H0], F16, kind="ExternalInput")
    w1s = nc.dram_tensor("w1s", [H0, H1], F32, kind="ExternalInput")
    w2s = nc.dram_tensor("w2s", [H1, NCLS], F32, kind="ExternalInput")
    b0c = nc.dram_tensor("b0c", [H0, 1], F32, kind="ExternalInput")
    b1c = nc.dram_tensor("b1c", [H1, 1], F32, kind="ExternalInput")
    b2r = nc.dram_tensor("b2r", [BL, NCLS], F32, kind="ExternalInput")
    out = nc.dram_tensor("out", [BL, NCLS], F32, kind="ExternalOutput")

    with _SplitDrainTileContext(nc) as tc:
        with tc.tile_pool(name="dram", bufs=1, space="DRAM") as dram:
            # A2A buffers: row r = 8*shard + bloc, K fastest (contiguous)
            zl = dram.tile([B, KSHP], F16)
            zex = dram.tile([B, KSHP], F16)
            cin = dram.tile([B, H0], F32)
            cout = dram.tile([BL, H0], F32)
            prow = dram.tile([1, W], F32)            # p[T-1] bounce

            with tc.tile_pool(name="const", bufs=1) as cpool, \
                 tc.tile_pool(name="w0p", bufs=1) as w0pool, \
                 tc.tile_pool(name="prep", bufs=1) as ppool:
                # ---- const loads ----
                xs_sb = cpool.tile([128, BL * IN_CH], F32)
                nc.scalar.dma_start(xs_sb[:], _ap(xs, 0, [BL * IN_CH, 128],
                                                  [1, BL * IN_CH]))
                cw_sb = cpool.tile([128, 16], F32)
                nc.scalar.dma_start(cw_sb[:], cwr[:])
                cb_sb = cpool.tile([128, OUT_CH], F32)
                nc.scalar.dma_start(cb_sb[:], cbr[:])
                tl_sb = cpool.tile([128, 1], F32)
                nc.scalar.dma_start(tl_sb[:], tlin[:])
                lt_sb = cpool.tile([128, 128], F16)
                nc.scalar.dma_start(lt_sb[:], ltri[:])
                onec_sb = cpool.tile([128, 1], F16)
                nc.scalar.dma_start(onec_sb[:], onec[:])
                idh_sb = cpool.tile([64, 64], F16)
                nc.scalar.dma_start(idh_sb[:], idh[:])
                onef_sb = cpool.tile([128, 1], F32)
                nc.scalar.dma_start(onef_sb[:], onef[:])
                idn_sb = cpool.tile([128, BL], F32)
                nc.scalar.dma_start(idn_sb[:], idn[:])
                w1_sb = [cpool.tile([128, H1], F32, tag=f"w1_{j}", name=f"w1_{j}")
                         for j in range(4)]
                for j in range(4):
                    nc.sync.dma_start(w1_sb[j][:], w1s[j * 128:(j + 1) * 128, :])
                w2_sb = [cpool.tile([128, NCLS], F32, tag=f"w2_{j}", name=f"w2_{j}")
                         for j in range(2)]
                for j in range(2):
                    nc.sync.dma_start(w2_sb[j][:], w2s[j * 128:(j + 1) * 128, :])
                b0_sb = cpool.tile([128, 4], F32)
                nc.scalar.dma_start(b0_sb[:], _ap(b0c, 0, [1, 128], [128, 4]))
                b1_sb = cpool.tile([128, 2], F32)
                nc.scalar.dma_start(b1_sb[:], _ap(b1c, 0, [1, 128], [128, 2]))
                b2_sb = cpool.tile([BL, NCLS], F32)
                nc.scalar.dma_start(b2_sb[:], b2r[:])
                # ---- w0 shard prefetch (streams during signature + A2A) ----
                w0t = [w0pool.tile([128, 14 * H0], F16, tag="w0a", name="w0a"),
                       w0pool.tile([128, 14 * H0], F16, tag="w0b", name="w0b"),
                       w0pool.tile([128, H0], F16, tag="w0c", name="w0c")]
                for h in range(2):
                    nc.sync.dma_start(
                        w0t[h][:],
                        _ap(w0s, h * 14 * 128 * H0, [H0, 128],
                            [128 * H0, 14], [1, H0]))
                nc.sync.dma_start(
                    w0t[2][:],
                    _ap(w0s, 28 * 128 * H0, [H0, 128], [1, H0]))

                # ---- prep: conv -> path p, then dx, P1, Y, ut4, u24, at ----
                # lane order oc-major: lane = oc*8 + bloc  (group g == oc g)
                p = ppool.tile([128, W], F32, tag="p")
                dx = ppool.tile([128, W], F32, tag="dx")
                p1 = ppool.tile([128, W], F32, tag="p1")
                yt = ppool.tile([128, W], F32, tag="yt")
                pl = ppool.tile([128, W], F32, tag="pl")
                ut = ppool.tile([128, W], F32, tag="ut")
                u2 = ppool.tile([128, W], F32, tag="u2")
                at = ppool.tile([128, W], F32, tag="at")
                tmpc = ppool.tile([128, BL * IN_CH], F32, tag="tmpc")
                s1h = ppool.tile([128, W], F16, tag="s1h")

                # conv: p[t, (oc,bloc,c)+1] = sum_k x[t, bloc, 4c+k] w[oc,k]
                pdst = _ap(p, 1, [W, 128], [GL * D, OUT_CH], [D, BL], [1, CH])
                tvw = _ap(tmpc, 0, [BL * IN_CH, 128],
                          [GL * CH, OUT_CH], [CH, BL], [1, CH])

                def xsv(k):
                    return _ap(xs_sb, k, [BL * IN_CH, 128],
                               [0, OUT_CH], [IN_CH, BL], [4, CH])

                def cwv(k):
                    return _ap(cw_sb, k, [16, 128], [4, OUT_CH], [0, BL],
                               [0, CH])

                # split the 4-tap conv across DVE (taps 0,1) / GPSIMD (2,3)
                nc.vector.tensor_tensor(pdst, xsv(0), cwv(0), AL.mult)
                nc.vector.tensor_tensor(tvw, xsv(1), cwv(1), AL.mult)
                nc.vector.tensor_tensor(pdst, pdst, tvw, AL.add)
                tmpg = ppool.tile([128, BL * IN_CH], F32, tag="tmpg")
                tmph = ppool.tile([128, BL * IN_CH], F32, tag="tmph")
                tgw = _ap(tmpg, 0, [BL * IN_CH, 128],
                          [GL * CH, OUT_CH], [CH, BL], [1, CH])
                tgw2 = _ap(tmph, 0, [BL * IN_CH, 128],
                           [GL * CH, OUT_CH], [CH, BL], [1, CH])
                nc.gpsimd.tensor_tensor(tgw, xsv(2), cwv(2), AL.mult)
                nc.gpsimd.tensor_tensor(tgw2, xsv(3), cwv(3), AL.mult)
                nc.gpsimd.tensor_tensor(tgw, tgw, tgw2, AL.add)
                nc.vector.tensor_tensor(pdst, pdst, tgw, AL.add)
                cbv = _ap(cb_sb, 0, [OUT_CH, 128], [1, OUT_CH], [0, BL],
                          [0, CH])
                nc.vector.tensor_tensor(pdst, pdst, cbv, AL.add)
                # time channel into col 0 of every lane
                nc.vector.tensor_copy(_ap(p, 0, [W, 128], [D, LANES]),
                                      _ap(tl_sb, 0, [1, 128], [0, LANES]))
                # P1 = p shifted down one step (DMA: compute engines cannot
                # address unaligned partition bases), then dx = p - P1.
                nc.gpsimd.memset(p1[0:1, :], 0.0)
                nc.gpsimd.dma_start(p1[1:128, :], p[0:127, :])
                nc.vector.tensor_tensor(dx[:], p[:], p1[:], AL.subtract)
                # Y[t] = p[T-1] - p[t]  (broadcast last row via DRAM bounce)
                nc.scalar.dma_start(prow[:], p[127:128, :])
                nc.scalar.dma_start(pl[:], _ap(prow, 0, [0, 128], [1, W]))
                nc.gpsimd.tensor_tensor(yt[:], pl[:], p[:], AL.subtract)
                # S1 row in fp16 (unscaled; w0 S1 rows are x1 host-side);
                # pl has p[T-1] broadcast on every partition -> read row 0
                nc.vector.tensor_copy(s1h[0:1, :], pl[0:1, :])
                # at = P1 + dx/2;  ut4 = (P1/2 + dx/6)/4;  u24 = (P1/3+dx/12)/4
                nc.vector.scalar_tensor_tensor(at[:], dx[:], 0.5, p1[:],
                                               AL.mult, AL.add)
                nc.vector.tensor_scalar(ut[:], p1[:], 0.125, None, AL.mult)
                nc.vector.scalar_tensor_tensor(ut[:], dx[:], 1.0 / 24, ut[:],
                                               AL.mult, AL.add)
                # (TS/STT are DVE-only; use ACT copy-with-scale + Pool adds)
                tmpu = ppool.tile([128, W], F32, tag="tmpu")
                nc.scalar.activation(u2[:], p1[:], AF.Copy, scale=1.0 / 12)
                nc.scalar.activation(tmpu[:], dx[:], AF.Copy, scale=1.0 / 48)
                nc.gpsimd.tensor_tensor(u2[:], u2[:], tmpu[:], AL.add)
                # pre-scaled dx copies (STT is 3D-max; outer products are 4D)
                dr = ppool.tile([128, W], F32, tag="dr")   # dx/4  (rx)
                dq = ppool.tile([128, W], F32, tag="dq")   # dx/8  (q2)
                nc.scalar.activation(dr[:], dx[:], AF.Copy, scale=0.25)
                nc.scalar.activation(dq[:], dx[:], AF.Copy, scale=0.125)

                # ---- per-group signature: g == out-channel ----
                with tc.tile_pool(name="grp", bufs=2) as gpool, \
                     tc.tile_pool(name="ps2", bufs=2, space="PSUM") as ps2p, \
                     tc.tile_pool(name="ptab", bufs=1, space="PSUM") as ptabp, \
                     tc.tile_pool(name="ps12", bufs=1, space="PSUM") as ps12p:
                    for g in range(NG):
                        off = g * GL * D  # col offset into the 288-wide tiles
                        GW = GL * 81      # 648

                        def o_ij(t, st=1):  # [lane, i(step), j(bcast)] view
                            return _ap(t, off, [W, 128], [D, GL], [st, D],
                                       [0, D])

                        def o_ji(t, st=1):  # [lane, i(bcast), j(step)] view
                            return _ap(t, off, [W, 128], [D, GL], [0, D],
                                       [st, D])

                        # m2[t,(l,ij)] = at_i dx_j  (unscaled, fp16)
                        m2 = gpool.tile([128, GW], F16, tag="m2")
                        m2v = _ap(m2, 0, [GW, 128], [81, GL], [D, D], [1, D])
                        nc.vector.tensor_tensor(m2v, o_ij(at), o_ji(dx),
                                                AL.mult)

                        # s2[t] = sum_{s<t} m2[s]   (fp32 PSUM)
                        s2 = ps2p.tile([128, GW], F32, tag="s2")
                        nc.tensor.matmul(s2[:, 0:512], lt_sb[:], m2[:, 0:512],
                                         start=True, stop=True)
                        nc.tensor.matmul(s2[:, 512:GW], lt_sb[:],
                                         m2[:, 512:GW], start=True, stop=True)
                        # S2 = sum_t m2[t]  (ones-column matmul, fp32 PSUM)
                        s12 = ps12p.tile([1, 1024], F32, tag="s12")
                        nc.tensor.matmul(s12[0:1, 0:512], onec_sb[:],
                                         m2[:, 0:512], start=True, stop=True)
                        nc.tensor.matmul(s12[0:1, 512:GW], onec_sb[:],
                                         m2[:, 512:GW], start=True, stop=True)
                        z12 = gpool.tile([1, GW], F16, tag="z12")
                        nc.scalar.activation(z12[0:1, :], s12[0:1, 0:GW],
                                             AF.Copy)

                        # bt = ut4 x dx + s2/4 ; t8 = u24 x dx + s2/4  (fp16)
                        bt = gpool.tile([128, GW], F16, tag="bt")
                        btv = _ap(bt, 0, [GW, 128], [81, GL], [D, D], [1, D])
                        nc.vector.tensor_tensor(btv, o_ij(ut), o_ji(dx),
                                                AL.mult)
                        nc.vector.scalar_tensor_tensor(bt[:], s2[:], 0.25,
                                                       bt[:], AL.mult, AL.add)
                        t8 = gpool.tile([128, GW], F16, tag="t8")
                        t8v = _ap(t8, 0, [GW, 128], [81, GL], [D, D], [1, D])
                        nc.vector.tensor_tensor(t8v, o_ij(u2), o_ji(dx),
                                                AL.mult)
                        nc.vector.scalar_tensor_tensor(t8[:], s2[:], 0.25,
                                                       t8[:], AL.mult, AL.add)

                        # q2 = (dx/8) x dx   (the /32 of dx x dx/2 /16... :
                        #  S4 needs t8^T (dx x dx)/2 /16-scale-> /32 overall,
                        #  bt/t8 carry 1/4 each, so q2 carries 1/8)
                        q2 = gpool.tile([128, GW], F16, tag="q2")
                        q2v = _ap(q2, 0, [GW, 128], [81, GL], [D, D], [1, D])
                        nc.gpsimd.tensor_tensor(q2v, o_ij(dq), o_ji(dx),
                                                AL.mult)
                        # rx = [(dx/4) x Y | dx/4]  (90 cols per lane)
                        rx = gpool.tile([128, GL * 90], F16, tag="rx")
                        rxv = _ap(rx, 0, [GL * 90, 128], [90, GL], [D, D],
                                  [1, D])
                        nc.gpsimd.tensor_tensor(rxv, o_ij(dr), o_ji(yt),
                                                AL.mult)
                        nc.gpsimd.tensor_copy(
                            _ap(rx, 81, [GL * 90, 128], [90, GL], [1, D]),
                            _ap(dr, off, [W, 128], [D, GL], [1, D]))

                        # tab[l] = bt_l^T rx_l (+) t8_l^T q2_l  -> [81, 90]
                        tab = ptabp.tile([128, 1024], F32, tag="tab")
                        for l in range(GL):
                            nc.tensor.matmul(
                                _ap(tab, 128 * l, [1024, 81], [1, 90]),
                                bt[:, l * 81:(l + 1) * 81],
                                rx[:, l * 90:(l + 1) * 90],
                                start=True, stop=False)
                            nc.tensor.matmul(
                                _ap(tab, 128 * l, [1024, 81], [1, 81]),
                                t8[:, l * 81:(l + 1) * 81],
                                q2[:, l * 81:(l + 1) * 81],
                                start=False, stop=True)
                        # evacuate group: [81, (bloc, 90)] fp16 (ACT engine)
                        zt4 = gpool.tile([81, GL * 90], F16, tag="zt4")
                        nc.scalar.activation(
                            _ap(zt4, 0, [GL * 90, 81], [90, GL], [1, 90]),
                            _ap(tab, 0, [1024, 81], [128, GL], [1, 90]),
                            AF.Copy)

                        # ---- pack this group's features into zl ----
                        # lane (oc=g, bloc) shard 2g:   rows 16g + bloc
                        #                   shard 2g+1: rows 16g + 8 + bloc
                        e1 = nc.sync if g % 2 == 0 else nc.scalar
                        e2 = nc.scalar if g % 2 == 0 else nc.sync
                        # S2 header (81 els at col 9)
                        e1.dma_start(
                            _ap(zl, (16 * g) * KSHP + 9, [KSHP, GL], [1, 81]),
                            z12[0:1, :])
                        # ij 0..39 -> shard 2g cols [90 + ij*90 + kl]
                        e1.dma_start(
                            _ap(zl, (16 * g) * KSHP + 90, [90, 40],
                                [KSHP, GL], [1, 90]),
                            _ap(zt4, 0, [GL * 90, 40], [90, GL], [1, 90]))
                        # ij 40..80 -> shard 2g+1 cols [(ij-40)*90 + kl]
                        e2.dma_start(
                            _ap(zl, (16 * g + 8) * KSHP, [90, 41],
                                [KSHP, GL], [1, 90]),
                            _ap(zt4, 40 * GL * 90, [GL * 90, 41],
                                [90, GL], [1, 90]))

                    # S1 headers for all lanes (9 els at col 0); s1h row 0
                    # natural col order (oc, bloc, d) matches the dst order
                    nc.sync.dma_start(
                        _ap(zl, 0, [16 * KSHP, OUT_CH], [KSHP, GL], [1, D]),
                        s1h[0:1, :])
                    # zero the 22-el xbar pad of every row
                    zpad = ppool.tile([1, B * (KSHP - KSH)], F16, tag="zpad")
                    nc.gpsimd.memset(zpad[0:1, :], 0.0)
                    nc.scalar.dma_start(
                        _ap(zl, KSH, [KSHP, B], [1, KSHP - KSH]),
                        zpad[0:1, :])

                    nc.gpsimd.collective_compute(
                        "AllToAll", AL.bypass,
                        replica_groups=[list(range(NCORES))],
                        ins=[zl[:].opt()], outs=[zex[:].opt()])

                # preload ACT tables during the A2A window (dead time)
                dum = ppool.tile([1, 4], F32, tag="dum")
                nc.scalar.activation(dum[0:1, 0:1], tl_sb[0:1, 0:1],
                                     AF.Sigmoid)
                nc.scalar.activation(dum[0:1, 1:2], tl_sb[0:1, 0:1], AF.Exp)
                nc.scalar.activation(dum[0:1, 2:3], onef_sb[0:1, 0:1], AF.Ln)

                # ---- z0 = z @ w0 partial over this core's K shard ----
                with tc.tile_pool(name="zt", bufs=1) as ztp, \
                     tc.tile_pool(name="ptr", bufs=2, space="PSUM") as ptrp, \
                     tc.tile_pool(name="pz0", bufs=1, space="PSUM") as pz0p, \
                     tc.tile_pool(name="ptail", bufs=1, space="PSUM") as ptl:
                    # one fully-contiguous batch-major gather (7.4 KB runs),
                    # then PE-transpose each [64, 128] block to [128 K, 64 b]
                    zb = ztp.tile([B, KSHP], F16, tag="zb", name="zb")
                    nc.sync.dma_start(zb[:], zex[:])
                    zT = [ztp.tile([128, 14 * B], F16, tag="zta", name="zta"),
                          ztp.tile([128, 14 * B], F16, tag="ztb", name="ztb"),
                          ztp.tile([128, B], F16, tag="ztc", name="ztc")]

                    def zt_chunk(i):
                        if i < 28:
                            return zT[i // 14][:, (i % 14) * B:
                                               (i % 14 + 1) * B]
                        return zT[2][:]

                    z0p = pz0p.tile([B, H0], F32, tag="z0p")
                    for i in range(NCH):
                        ptr = ptrp.tile([128, B], F16, tag="ptr")
                        nc.tensor.transpose(ptr[:],
                                            zb[:, i * 128:(i + 1) * 128],
                                            idh_sb[:])
                        eng = nc.vector if i % 2 == 0 else nc.scalar
                        if i % 2 == 0:
                            eng.tensor_copy(zt_chunk(i), ptr[:])
                        else:
                            eng.activation(zt_chunk(i), ptr[:], AF.Copy)
                        if i < 28:
                            rhs = w0t[i // 14][:, (i % 14) * H0:
                                               (i % 14 + 1) * H0]
                        else:
                            rhs = w0t[2][:]
                        nc.tensor.matmul(z0p[:], zt_chunk(i), rhs,
                                         start=(i == 0), stop=(i == NCH - 1))
                    z0sb = cpool.tile([B, H0], F32, tag="z0sb", name="z0sb")
                    nc.vector.tensor_copy(z0sb[:], z0p[:])
                    nc.sync.dma_start(cin[:], z0sb[:])
                    nc.gpsimd.collective_compute(
                        "ReduceScatter", AL.add,
                        replica_groups=[list(range(NCORES))],
                        ins=[cin[:].opt()], outs=[cout[:].opt()])

                    # ---- tail: sigmoid(z0) -> w1 -> sigmoid -> w2 ----
                    z1row = cpool.tile([BL, H0], F32, tag="z1row")
                    nc.sync.dma_start(z1row[:], cout[:])
                    pz1 = ptl.tile([128, 4 * BL], F32, tag="pz1")
                    z1t = cpool.tile([128, 4 * BL], F32, tag="z1t")
                    for j in range(4):
                        nc.tensor.transpose(pz1[:, j * BL:(j + 1) * BL],
                                            z1row[:, j * 128:(j + 1) * 128],
                                            idn_sb[0:BL, 0:BL])
                        nc.scalar.activation(z1t[:, j * BL:(j + 1) * BL],
                                             pz1[:, j * BL:(j + 1) * BL],
                                             AF.Sigmoid, bias=b0_sb[:, j:j + 1])
                    pz2 = ptl.tile([128, 2 * BL], F32, tag="pz2")
                    z2t = cpool.tile([128, 2 * BL], F32, tag="z2t")
                    for m in range(2):
                        for kj in range(4):
                            nc.tensor.matmul(
                                pz2[:, m * BL:(m + 1) * BL],
                                w1_sb[kj][:, m * 128:(m + 1) * 128],
                                z1t[:, kj * BL:(kj + 1) * BL],
                                start=(kj == 0), stop=(kj == 3))
                        nc.scalar.activation(z2t[:, m * BL:(m + 1) * BL],
                                             pz2[:, m * BL:(m + 1) * BL],
                                             AF.Sigmoid, bias=b1_sb[:, m:m + 1])
                    pz3 = ptl.tile([BL, NCLS], F32, tag="pz3")
                    for m in range(2):
                        nc.tensor.matmul(pz3[:], z2t[:, m * BL:(m + 1) * BL],
                                         w2_sb[m][:], start=(m == 0),
                                         stop=(m == 1))
                    z3 = cpool.tile([BL, NCLS], F32, tag="z3")
                    nc.vector.tensor_tensor(z3[:], pz3[:], b2_sb[:], AL.add)
                    mx = cpool.tile([BL, 1], F32, tag="mx")
                    nc.vector.tensor_reduce(mx[:], z3[:], mybir.AxisListType.X,
                                            AL.max)
                    tm = cpool.tile([BL, NCLS], F32, tag="tm")
                    nc.vector.tensor_scalar(tm[:], z3[:], mx[:, 0:1], None,
                                            AL.subtract)
                    ex = cpool.tile([BL, NCLS], F32, tag="ex")
                    se = cpool.tile([BL, 1], F32, tag="se")
                    nc.scalar.activation(ex[:], tm[:], AF.Exp, accum_out=se[:])
                    ls = cpool.tile([BL, 1], F32, tag="ls")
                    nc.scalar.activation(ls[:], se[:], AF.Ln)
                    osb = cpool.tile([BL, NCLS], F32, tag="osb")
                    nc.vector.tensor_scalar(osb[:], tm[:], ls[:, 0:1], None,
                                            AL.subtract)
                    nc.sync.dma_start(out[:], osb[:])
    return nc


_CACHE = {}


def kernel(x, conv_w, conv_b, w0, b0, w1, b1, w2, b2):
    x = np.ascontiguousarray(np.asarray(x, np.float32))
    conv_w = np.asarray(conv_w, np.float32)
    conv_b = np.asarray(conv_b, np.float32)
    w0 = np.asarray(w0, np.float32)
    w1 = np.ascontiguousarray(np.asarray(w1, np.float32))
    w2 = np.ascontiguousarray(np.asarray(w2, np.float32))
    b0 = np.asarray(b0, np.float32)
    b1 = np.asarray(b1, np.float32)
    b2 = np.asarray(b2, np.float32)

    if "nc" not in _CACHE:
        _CACHE["nc"] = _build()
        _CACHE["perm"] = _w0_perm()
        # S3/S4 features arrive /4; S1/S2 natural scale
        sc = np.ones(SIGC, np.float32)
        sc[90:] = FSCALE
        _CACHE["wscale"] = np.tile(sc, OUT_CH)
    nc = _CACHE["nc"]
    w0p = (w0[_CACHE["perm"], :]
           * _CACHE["wscale"][:, None]).astype(np.float16)
    w0pp = np.zeros((NCORES, KSHP, H0), np.float16)
    w0pp[:, :KSH, :] = w0p.reshape(NCORES, KSH, H0)

    shared = {
        "cwr": np.ascontiguousarray(
            np.broadcast_to(conv_w.reshape(1, 16), (128, 16))),
        "cbr": np.ascontiguousarray(
            np.broadcast_to(conv_b.reshape(1, OUT_CH), (128, OUT_CH))),
        "tlin": np.linspace(0.0, 1.0, T, dtype=np.float32).reshape(128, 1),
        "ltri": np.ascontiguousarray(
            np.triu(np.ones((128, 128), np.float32), 1)).astype(np.float16),
        "onec": np.ones((128, 1), np.float16),
        "idh": np.ascontiguousarray(np.eye(64, dtype=np.float16)),
        "onef": np.ones((128, 1), np.float32),
        "idn": np.ascontiguousarray(np.eye(128, BL, dtype=np.float32)),
        "w1s": w1, "w2s": w2,
        "b0c": b0.reshape(H0, 1), "b1c": b1.reshape(H1, 1),
        "b2r": np.ascontiguousarray(np.broadcast_to(b2.reshape(1, NCLS),
                                                    (BL, NCLS))),
    }
    in_maps = []
    for c in range(NCORES):
        m = dict(shared)
        m["xs"] = np.ascontiguousarray(
            x[c * BL:(c + 1) * BL, 0].transpose(1, 0, 2))
        m["w0s"] = np.ascontiguousarray(w0pp[c])
        in_maps.append(m)

    _CACHE["in_maps"] = in_maps
    res = run_bass_kernel_spmd(nc, in_maps, core_ids=list(range(NCORES)))
    return np.concatenate([res.results[c]["out"] for c in range(NCORES)],
                          axis=0)


# revision 39
# speedup vs baseline: 1.0100x; 1.0100x over previous
"""CNN + truncated path-signature (depth 4) + FF head on 8 TRN2 NeuronCores.

Strategy
--------
- Batch data-parallel signature computation: core c handles batches
  [8c, 8c+8) = 32 (batch, out_ch) lanes, T=128 time steps on partitions.
- Signature reformulated to avoid sequential scans:
    dx, P1 (= shifted path), Y (= suffix sums) are free;
    the only prefix scan (level 2, s2) is one triangular matmul L @ m2;
    S3 = bt^T dx;  S4 = bt^T (dx x Y) + t8^T (dx x dx)/2,
  so levels 3 and 4 are plain T-contractions on the TensorEngine.
- All PE contractions run in fp16 (1 cycle/row; fp32 modes are 4x slower),
  accumulating in fp32 PSUM.  fp16's 10-bit mantissa keeps the final
  rel-err ~2.5e-3 (bf16 operands measured 1.3e-2, too close to the gate).
- w0 (60 MB fp32) is row-sharded 8 ways as fp16 (3.8 MB/core stream):
  AllToAll re-shards the fp16 signature activations feature-wise, each
  core multiplies its [3690, 512] w0 shard for all 64 batches, and a
  fp16 ReduceScatter returns each core its own 8 batches.
- A2A buffer layout is [batch, K] (K fastest) so both the pack DMAs
  (180 B runs) and the post-A2A lhsT gathers (256 B runs) avoid the
  16-byte-run descriptor storm the K-major layout suffers.
- w0 rows are permuted and pre-scaled host-side (S3/S4 features are
  produced /4 on device to keep fp16 headroom), so no on-device
  transposes or rescales are needed.
"""
import os
import sys
sys.path.insert(0, "/opt/trn_rl_repo")
if os.environ.get("JAX_PLATFORMS") == "cpu":
    # The SPMD launch needs the axon/neuron PJRT backend.
    os.environ["JAX_PLATFORMS"] = ""

import numpy as np
import bass_rust as _bass_rust
import concourse.bass as bass
import concourse.tile as tile
import concourse.mybir as mybir
from concourse.vector_clock import ScopedClock
from concourse.bass_utils import run_bass_kernel_spmd

F32 = mybir.dt.float32
F16 = mybir.dt.float16
AL = mybir.AluOpType
AF = mybir.ActivationFunctionType

NCORES = 8
B, T, IN_CH = 64, 128, 32
OUT_CH, CH, D = 4, 8, 9            # conv out-channels, conv width, path dim
BL = B // NCORES                   # local batches = 8
LANES = BL * OUT_CH                # 32 lanes/core
NG = 4                             # lane groups == out-channels
GL = 8                             # lanes per group == local batches
SIGC = 7380                        # per-lane signature channels
KSH = OUT_CH * SIGC // NCORES      # w0 K-shard rows per core = 3690
NCH = 29                           # K chunks of 128
KSHP = NCH * 128                   # shard padded to 3712 (xbar %128 rule)
H0, H1, NCLS = 512, 256, 10
FSCALE = 4.0                       # S3/S4 features arrive /4 (fp16 headroom)
W = LANES * D                      # 288


class _SplitDrainTileContext(tile.TileContext):
    """Tile exit drain carries one sem wait per CTRL instruction.

    This container's walrus build rejects >2 sync waits on a CTRL
    instruction; Tile's exit drain waits on the whole global clock.
    Redistribute the waits over nops on the same engine (program order on
    one engine preserves semantics)."""

    MAX_WAITS = 1

    def _split_body_waits(self):
        """Move excess sem waits from any instruction onto preceding nops on
        the same engine (same-engine program order preserves semantics)."""
        nc = self.nc
        for bb in nc.main_func.blocks:
            heavy = [ins for ins in bb.instructions
                     if ins.sync_info and ins.sync_info.on_wait
                     and len(ins.sync_info.on_wait) > self.MAX_WAITS]
            if not heavy:
                continue
            extra = {}
            for ins in heavy:
                w = list(ins.sync_info.on_wait)
                ins.sync_info.on_wait = w[:self.MAX_WAITS]
                nops = []
                for i in range(self.MAX_WAITS, len(w), self.MAX_WAITS):
                    n = nc.engines[ins.engine].nop(hint="wait_split")
                    # pop the freshly appended nop from wherever it landed
                    for bb2 in nc.main_func.blocks:
                        if bb2.instructions and bb2.instructions[-1] is n.ins:
                            bb2.instructions.pop()
                            break
                    for wt in w[i:i + self.MAX_WAITS]:
                        handle = _bass_rust.SemaphoreHandle(wt.ant_name, wt.id)
                        _bass_rust.wait_op(n.ins, handle, wt.wait_value,
                                           "sem-ge", False)
                    nops.append(n.ins)
                extra[id(ins)] = nops
            new_list = []
            for ins in bb.instructions:
                new_list.extend(extra.get(id(ins), ()))
                new_list.append(ins)
            bb.instructions[:] = new_list

    def _drain_and_barrier(self, tick_clock, wait_clock):
        nc = self.nc
        self._split_body_waits()
        probe = nc.sync.nop(hint="tile_exit_wait_0")
        wait_clock.add_sem_waits(
            probe.ins, ScopedClock({None: tick_clock.global_clock})
        )
        waits = list(probe.ins.sync_info.on_wait or [])
        probe.ins.sync_info.on_wait = waits[:1]
        for w in waits[1:]:
            n = nc.sync.nop(hint="tile_exit_wait")
            handle = _bass_rust.SemaphoreHandle(w.ant_name, w.id)
            _bass_rust.wait_op(n.ins, handle, w.wait_value, "sem-ge", False)
        nc.sync.drain()
        nc.all_engine_barrier()
        assert self.sems is not None
        popped = nc._tile_sem_poison_stack.pop()
        assert popped is self._sem_poison
        nc.clear_and_free_semaphores(list(self.sems.allocated().values()))
        nc.all_engine_barrier()


def _ap(t, extra, *dims):
    """AP over tile t's buffer: partition dim from the tile, custom free dims.

    dims[0] may override the partition [step, count]."""
    base = t[:]
    return bass.AP(base.tensor, base.offset + extra, list(dims))


def _w0_perm():
    """Row permutation p s.t. w0_permuted[i] = w0[p[i]] matches the kernel's
    feature order: per oc: [S1(9) | S2(81)] then rows 90 + ij*90 + c with
    c<81 -> level4 (ij,kl=c), c>=81 -> level3 (ij, k=c-81)."""
    p = np.empty(OUT_CH * SIGC, np.int64)
    i = 0
    for oc in range(OUT_CH):
        base = oc * SIGC
        p[i:i + 90] = base + np.arange(90)          # S1 then S2, native order
        i += 90
        for ij in range(81):
            # level-4 block (ij, kl) for kl in 0..80 -> orig 819 + ij*81 + kl
            p[i:i + 81] = base + 819 + ij * 81 + np.arange(81)
            i += 81
            # level-3 (ij, k) for k in 0..8 -> orig 90 + ij*9 + k
            p[i:i + 9] = base + 90 + ij * 9 + np.arange(9)
            i += 9
    assert i == OUT_CH * SIGC
    return p


def _build():
    nc = bass.Bass(num_devices=NCORES, target_bir_lowering=True, trn_type="TRN2")

    # ---- per-core DRAM inputs ----
    xs = nc.dram_tensor("xs", [T, BL, IN_CH], F32, kind="ExternalInput")
    cwr = nc.dram_tensor("cwr", [128, 16], F32, kind="ExternalInput")
    cbr = nc.dram_tensor("cbr", [128, OUT_CH], F32, kind="ExternalInput")
    tlin = nc.dram_tensor("tlin", [128, 1], F32, kind="ExternalInput")
    ltri = nc.dram_tensor("ltri", [128, 128], F16, kind="ExternalInput")
    onec = nc.dram_tensor("onec", [128, 1], F16, kind="ExternalInput")
    idh = nc.dram_tensor("idh", [64, 64], F16, kind="ExternalInput")
    onef = nc.dram_tensor("onef", [128, 1], F32, kind="ExternalInput")
    idn = nc.dram_tensor("idn", [128, BL], F32, kind="ExternalInput")
    w0s = nc.dram_tensor("w0s", [KSHP, H0], F16, kind="ExternalInput")
    w1s = nc.dram_tensor("w1s", [H0, H1], F16, kind="ExternalInput")
    w2s = nc.dram_tensor("w2s", [H1, NCLS], F16, kind="ExternalInput")
    b0c = nc.dram_tensor("b0c", [H0, 1], F32, kind="ExternalInput")
    b1c = nc.dram_tensor("b1c", [H1, 1], F32, kind="ExternalInput")
    b2r = nc.dram_tensor("b2r", [BL, NCLS], F32, kind="ExternalInput")
    out = nc.dram_tensor("out", [BL, NCLS], F32, kind="ExternalOutput")

    with _SplitDrainTileContext(nc) as tc:
        with tc.tile_pool(name="dram", bufs=1, space="DRAM") as dram:
            # A2A buffers: row r = 8*shard + bloc, K fastest (contiguous)
            zl = dram.tile([B, KSHP], F16)
            zex = dram.tile([B, KSHP], F16)
            cin = dram.tile([B, H0], F32)
            cout = dram.tile([BL, H0], F32)
            prow = dram.tile([1, W], F32)            # p[T-1] bounce

            with tc.tile_pool(name="const", bufs=1) as cpool, \
                 tc.tile_pool(name="w0p", bufs=1) as w0pool, \
                 tc.tile_pool(name="prep", bufs=1) as ppool:
                # ---- const loads ----
                xs_sb = cpool.tile([128, BL * IN_CH], F32)
                nc.scalar.dma_start(xs_sb[:], _ap(xs, 0, [BL * IN_CH, 128],
                                                  [1, BL * IN_CH]))
                cw_sb = cpool.tile([128, 16], F32)
                nc.scalar.dma_start(cw_sb[:], cwr[:])
                cb_sb = cpool.tile([128, OUT_CH], F32)
                nc.scalar.dma_start(cb_sb[:], cbr[:])
                tl_sb = cpool.tile([128, 1], F32)
                nc.scalar.dma_start(tl_sb[:], tlin[:])
                lt_sb = cpool.tile([128, 128], F16)
                nc.scalar.dma_start(lt_sb[:], ltri[:])
                onec_sb = cpool.tile([128, 1], F16)
                nc.scalar.dma_start(onec_sb[:], onec[:])
                idh_sb = cpool.tile([64, 64], F16)
                nc.scalar.dma_start(idh_sb[:], idh[:])
                onef_sb = cpool.tile([128, 1], F32)
                nc.scalar.dma_start(onef_sb[:], onef[:])
                idn_sb = cpool.tile([128, BL], F32)
                nc.scalar.dma_start(idn_sb[:], idn[:])
                w1_sb = [cpool.tile([128, H1], F16, tag=f"w1_{j}", name=f"w1_{j}")
                         for j in range(4)]
                for j in range(4):
                    nc.sync.dma_start(w1_sb[j][:], w1s[j * 128:(j + 1) * 128, :])
                w2_sb = [cpool.tile([128, NCLS], F16, tag=f"w2_{j}", name=f"w2_{j}")
                         for j in range(2)]
                for j in range(2):
                    nc.sync.dma_start(w2_sb[j][:], w2s[j * 128:(j + 1) * 128, :])
                b0_sb = cpool.tile([128, 4], F32)
                nc.scalar.dma_start(b0_sb[:], _ap(b0c, 0, [1, 128], [128, 4]))
                b1_sb = cpool.tile([128, 2], F32)
                nc.scalar.dma_start(b1_sb[:], _ap(b1c, 0, [1, 128], [128, 2]))
                b2_sb = cpool.tile([BL, NCLS], F32)
                nc.scalar.dma_start(b2_sb[:], b2r[:])
                # ---- w0 shard prefetch (streams during signature + A2A) ----
                w0t = [w0pool.tile([128, 14 * H0], F16, tag="w0a", name="w0a"),
                       w0pool.tile([128, 14 * H0], F16, tag="w0b", name="w0b"),
                       w0pool.tile([128, H0], F16, tag="w0c", name="w0c")]
                for h in range(2):
                    nc.sync.dma_start(
                        w0t[h][:],
                        _ap(w0s, h * 14 * 128 * H0, [H0, 128],
                            [128 * H0, 14], [1, H0]))
                nc.sync.dma_start(
                    w0t[2][:],
                    _ap(w0s, 28 * 128 * H0, [H0, 128], [1, H0]))

                # ---- prep: conv -> path p, then dx, P1, Y, ut4, u24, at ----
                # lane order oc-major: lane = oc*8 + bloc  (group g == oc g)
                p = ppool.tile([128, W], F32, tag="p")
                dx = ppool.tile([128, W], F32, tag="dx")
                p1 = ppool.tile([128, W], F32, tag="p1")
                yt = ppool.tile([128, W], F32, tag="yt")
                pl = ppool.tile([128, W], F32, tag="pl")
                at = ppool.tile([128, W], F32, tag="at")
                tmpc = ppool.tile([128, BL * IN_CH], F32, tag="tmpc")
                s1h = ppool.tile([128, W], F16, tag="s1h")

                # conv: p[t, (oc,bloc,c)+1] = sum_k x[t, bloc, 4c+k] w[oc,k]
                pdst = _ap(p, 1, [W, 128], [GL * D, OUT_CH], [D, BL], [1, CH])
                tvw = _ap(tmpc, 0, [BL * IN_CH, 128],
                          [GL * CH, OUT_CH], [CH, BL], [1, CH])

                def xsv(k):
                    return _ap(xs_sb, k, [BL * IN_CH, 128],
                               [0, OUT_CH], [IN_CH, BL], [4, CH])

                def cwv(k):
                    return _ap(cw_sb, k, [16, 128], [4, OUT_CH], [0, BL],
                               [0, CH])

                # split the 4-tap conv across DVE (taps 0,1) / GPSIMD (2,3)
                nc.vector.tensor_tensor(pdst, xsv(0), cwv(0), AL.mult)
                nc.vector.tensor_tensor(tvw, xsv(1), cwv(1), AL.mult)
                nc.vector.tensor_tensor(pdst, pdst, tvw, AL.add)
                tmpg = ppool.tile([128, BL * IN_CH], F32, tag="tmpg")
                tmph = ppool.tile([128, BL * IN_CH], F32, tag="tmph")
                tgw = _ap(tmpg, 0, [BL * IN_CH, 128],
                          [GL * CH, OUT_CH], [CH, BL], [1, CH])
                tgw2 = _ap(tmph, 0, [BL * IN_CH, 128],
                           [GL * CH, OUT_CH], [CH, BL], [1, CH])
                nc.gpsimd.tensor_tensor(tgw, xsv(2), cwv(2), AL.mult)
                nc.gpsimd.tensor_tensor(tgw2, xsv(3), cwv(3), AL.mult)
                nc.gpsimd.tensor_tensor(tgw, tgw, tgw2, AL.add)
                nc.vector.tensor_tensor(pdst, pdst, tgw, AL.add)
                cbv = _ap(cb_sb, 0, [OUT_CH, 128], [1, OUT_CH], [0, BL],
                          [0, CH])
                nc.vector.tensor_tensor(pdst, pdst, cbv, AL.add)
                # time channel into col 0 of every lane
                nc.vector.tensor_copy(_ap(p, 0, [W, 128], [D, LANES]),
                                      _ap(tl_sb, 0, [1, 128], [0, LANES]))
                # P1 = p shifted down one step (DMA: compute engines cannot
                # address unaligned partition bases), then dx = p - P1.
                nc.gpsimd.memset(p1[0:1, :], 0.0)
                nc.gpsimd.dma_start(p1[1:128, :], p[0:127, :])
                nc.vector.tensor_tensor(dx[:], p[:], p1[:], AL.subtract)
                # Y[t] = p[T-1] - p[t]  (broadcast last row via DRAM bounce)
                nc.scalar.dma_start(prow[:], p[127:128, :])
                nc.scalar.dma_start(pl[:], _ap(prow, 0, [0, 128], [1, W]))
                nc.gpsimd.tensor_tensor(yt[:], pl[:], p[:], AL.subtract)
                # S1 row in fp16 (unscaled; w0 S1 rows are x1 host-side);
                # pl has p[T-1] broadcast on every partition -> read row 0
                nc.vector.tensor_copy(s1h[0:1, :], pl[0:1, :])
                # at = P1 + dx/2 (for m2)
                nc.vector.scalar_tensor_tensor(at[:], dx[:], 0.5, p1[:],
                                               AL.mult, AL.add)
                # pre-scaled dx copies (STT is 3D-max; outer products are 4D)
                dr = ppool.tile([128, W], F32, tag="dr")   # dx/4  (rx)
                dq = ppool.tile([128, W], F32, tag="dq")   # dx/8  (q2)
                nc.scalar.activation(dr[:], dx[:], AF.Copy, scale=0.25)
                nc.scalar.activation(dq[:], dx[:], AF.Copy, scale=0.125)

                # ---- per-group signature: g == out-channel ----
                with tc.tile_pool(name="grp", bufs=2) as gpool, \
                     tc.tile_pool(name="ps2", bufs=2, space="PSUM") as ps2p, \
                     tc.tile_pool(name="ptab", bufs=1, space="PSUM") as ptabp, \
                     tc.tile_pool(name="ps12", bufs=1, space="PSUM") as ps12p:
                    for g in range(NG):
                        off = g * GL * D  # col offset into the 288-wide tiles
                        GW = GL * 81      # 648

                        def o_ij(t, st=1):  # [lane, i(step), j(bcast)] view
                            return _ap(t, off, [W, 128], [D, GL], [st, D],
                                       [0, D])

                        def o_ji(t, st=1):  # [lane, i(bcast), j(step)] view
                            return _ap(t, off, [W, 128], [D, GL], [0, D],
                                       [st, D])

                        # m2[t,(l,ij)] = at_i dx_j  (unscaled, fp16)
                        m2 = gpool.tile([128, GW], F16, tag="m2")
                        m2v = _ap(m2, 0, [GW, 128], [81, GL], [D, D], [1, D])
                        nc.vector.tensor_tensor(m2v, o_ij(at), o_ji(dx),
                                                AL.mult)

                        # s2[t] = sum_{s<t} m2[s]   (fp32 PSUM)
                        s2 = ps2p.tile([128, GW], F32, tag="s2")
                        nc.tensor.matmul(s2[:, 0:512], lt_sb[:], m2[:, 0:512],
                                         start=True, stop=True)
                        nc.tensor.matmul(s2[:, 512:GW], lt_sb[:],
                                         m2[:, 512:GW], start=True, stop=True)
                        # S2 = sum_t m2[t]  (ones-column matmul, fp32 PSUM)
                        s12 = ps12p.tile([1, 1024], F32, tag="s12")
                        nc.tensor.matmul(s12[0:1, 0:512], onec_sb[:],
                                         m2[:, 0:512], start=True, stop=True)
                        nc.tensor.matmul(s12[0:1, 512:GW], onec_sb[:],
                                         m2[:, 512:GW], start=True, stop=True)
                        z12 = gpool.tile([1, GW], F16, tag="z12")
                        nc.scalar.activation(z12[0:1, :], s12[0:1, 0:GW],
                                             AF.Copy)

                        # q2 = (dx/8) x dx  (= q2_true/4; fp16)
                        q2 = gpool.tile([128, GW], F16, tag="q2")
                        q2v = _ap(q2, 0, [GW, 128], [81, GL], [D, D], [1, D])
                        nc.gpsimd.tensor_tensor(q2v, o_ij(dq), o_ji(dx),
                                                AL.mult)
                        # bt/t8 from m2/q2/s2 algebra (contiguous 2D ops):
                        #   bt = m2/2 - (2/3) q2 + s2 ; t8 = m2/3 - (2/3)q2+s2
                        yc = gpool.tile([128, GW], F16, tag="yc")
                        nc.vector.scalar_tensor_tensor(yc[:], q2[:],
                                                       -2.0 / 3, s2[:],
                                                       AL.mult, AL.add)
                        bt = gpool.tile([128, GW], F16, tag="bt")
                        nc.vector.scalar_tensor_tensor(bt[:], m2[:], 0.5,
                                                       yc[:], AL.mult, AL.add)
                        t8 = gpool.tile([128, GW], F16, tag="t8")
                        nc.vector.scalar_tensor_tensor(t8[:], m2[:], 1.0 / 3,
                                                       yc[:], AL.mult, AL.add)

                        # rx = [(dx/4) x Y | dx/4]  (90 cols per lane)
                        rx = gpool.tile([128, GL * 90], F16, tag="rx")
                        rxv = _ap(rx, 0, [GL * 90, 128], [90, GL], [D, D],
                                  [1, D])
                        nc.vector.tensor_tensor(rxv, o_ij(dr), o_ji(yt),
                                                AL.mult)
                        nc.gpsimd.tensor_copy(
                            _ap(rx, 81, [GL * 90, 128], [90, GL], [1, D]),
                            _ap(dr, off, [W, 128], [D, GL], [1, D]))

                        # tab[l] = bt_l^T rx_l (+) t8_l^T q2_l  -> [81, 90]
                        tab = ptabp.tile([128, 1024], F32, tag="tab")
                        for l in range(GL):
                            nc.tensor.matmul(
                                _ap(tab, 128 * l, [1024, 81], [1, 90]),
                                bt[:, l * 81:(l + 1) * 81],
                                rx[:, l * 90:(l + 1) * 90],
                                start=True, stop=False)
                            nc.tensor.matmul(
                                _ap(tab, 128 * l, [1024, 81], [1, 81]),
                                t8[:, l * 81:(l + 1) * 81],
                                q2[:, l * 81:(l + 1) * 81],
                                start=False, stop=True)
                        # evacuate group: [81, (bloc, 90)] fp16 (ACT engine)
                        zt4 = gpool.tile([81, GL * 90], F16, tag="zt4")
                        nc.scalar.activation(
                            _ap(zt4, 0, [GL * 90, 81], [90, GL], [1, 90]),
                            _ap(tab, 0, [1024, 81], [128, GL], [1, 90]),
                            AF.Copy)

                        # ---- pack this group's features into zl ----
                        # lane (oc=g, bloc) shard 2g:   rows 16g + bloc
                        #                   shard 2g+1: rows 16g + 8 + bloc
                        e1 = nc.sync if g % 2 == 0 else nc.scalar
                        e2 = nc.scalar if g % 2 == 0 else nc.sync
                        # S2 header (81 els at col 9)
                        e1.dma_start(
                            _ap(zl, (16 * g) * KSHP + 9, [KSHP, GL], [1, 81]),
                            z12[0:1, :])
                        # ij 0..39 -> shard 2g cols [90 + ij*90 + kl]
                        e1.dma_start(
                            _ap(zl, (16 * g) * KSHP + 90, [90, 40],
                                [KSHP, GL], [1, 90]),
                            _ap(zt4, 0, [GL * 90, 40], [90, GL], [1, 90]))
                        # ij 40..80 -> shard 2g+1 cols [(ij-40)*90 + kl]
                        e2.dma_start(
                            _ap(zl, (16 * g + 8) * KSHP, [90, 41],
                                [KSHP, GL], [1, 90]),
                            _ap(zt4, 40 * GL * 90, [GL * 90, 41],
                                [90, GL], [1, 90]))

                    # S1 headers for all lanes (9 els at col 0); s1h row 0
                    # natural col order (oc, bloc, d) matches the dst order
                    nc.sync.dma_start(
                        _ap(zl, 0, [16 * KSHP, OUT_CH], [KSHP, GL], [1, D]),
                        s1h[0:1, :])
                    # zero the 22-el xbar pad of every row
                    zpad = ppool.tile([1, B * (KSHP - KSH)], F16, tag="zpad")
                    nc.gpsimd.memset(zpad[0:1, :], 0.0)
                    nc.scalar.dma_start(
                        _ap(zl, KSH, [KSHP, B], [1, KSHP - KSH]),
                        zpad[0:1, :])

                    nc.gpsimd.collective_compute(
                        "AllToAll", AL.bypass,
                        replica_groups=[list(range(NCORES))],
                        ins=[zl[:].opt()], outs=[zex[:].opt()])

                # ---- z0 = z @ w0 partial over this core's K shard ----
                with tc.tile_pool(name="zt", bufs=1) as ztp, \
                     tc.tile_pool(name="ptr", bufs=2, space="PSUM") as ptrp, \
                     tc.tile_pool(name="pz0", bufs=1, space="PSUM") as pz0p, \
                     tc.tile_pool(name="ptail", bufs=1, space="PSUM") as ptl:
                    # one fully-contiguous batch-major gather (7.4 KB runs),
                    # then PE-transpose each [64, 128] block to [128 K, 64 b]
                    zb = ztp.tile([B, KSHP], F16, tag="zb", name="zb")
                    nc.sync.dma_start(zb[:], zex[:])
                    zT = [ztp.tile([128, 14 * B], F16, tag="zta", name="zta"),
                          ztp.tile([128, 14 * B], F16, tag="ztb", name="ztb"),
                          ztp.tile([128, B], F16, tag="ztc", name="ztc")]

                    def zt_chunk(i):
                        if i < 28:
                            return zT[i // 14][:, (i % 14) * B:
                                               (i % 14 + 1) * B]
                        return zT[2][:]

                    # all transposes first (HAM ignores transpose-mode, so
                    # interleaving would keep the PE cold), then a dense
                    # matmul burst that warms up and stays warm
                    for i in range(NCH):
                        ptr = ptrp.tile([128, B], F16, tag="ptr")
                        nc.tensor.transpose(ptr[:],
                                            zb[:, i * 128:(i + 1) * 128],
                                            idh_sb[:])
                        if i % 2 == 0:
                            nc.vector.tensor_copy(zt_chunk(i), ptr[:])
                        else:
                            nc.scalar.activation(zt_chunk(i), ptr[:], AF.Copy)
                    z0p = pz0p.tile([B, H0], F32, tag="z0p")
                    for i in range(NCH):
                        if i < 28:
                            rhs = w0t[i // 14][:, (i % 14) * H0:
                                               (i % 14 + 1) * H0]
                        else:
                            rhs = w0t[2][:]
                        nc.tensor.matmul(z0p[:], zt_chunk(i), rhs,
                                         start=(i == 0), stop=(i == NCH - 1))
                    z0sb = cpool.tile([B, H0], F32, tag="z0sb", name="z0sb")
                    nc.vector.tensor_copy(z0sb[:], z0p[:])
                    nc.sync.dma_start(cin[:], z0sb[:])
                    nc.gpsimd.collective_compute(
                        "ReduceScatter", AL.add,
                        replica_groups=[list(range(NCORES))],
                        ins=[cin[:].opt()], outs=[cout[:].opt()])

                    # preload ACT tables during the RS window (dead time,
                    # and after the last AF.Copy use so nothing re-evicts)
                    dum = ppool.tile([1, 4], F32, tag="dum")
                    nc.scalar.activation(dum[0:1, 0:1], tl_sb[0:1, 0:1],
                                         AF.Sigmoid)
                    nc.scalar.activation(dum[0:1, 1:2], tl_sb[0:1, 0:1],
                                         AF.Exp)
                    nc.scalar.activation(dum[0:1, 2:3], onef_sb[0:1, 0:1],
                                         AF.Ln)

                    # ---- tail: sigmoid(z0) -> w1 -> sigmoid -> w2 ----
                    z1row = cpool.tile([BL, H0], F32, tag="z1row")
                    nc.sync.dma_start(z1row[:], cout[:])
                    pz1 = ptl.tile([128, 4 * BL], F32, tag="pz1")
                    z1t = cpool.tile([128, 4 * BL], F16, tag="z1t")
                    for j in range(4):
                        nc.tensor.transpose(pz1[:, j * BL:(j + 1) * BL],
                                            z1row[:, j * 128:(j + 1) * 128],
                                            idn_sb[0:BL, 0:BL])
                        nc.scalar.activation(z1t[:, j * BL:(j + 1) * BL],
                                             pz1[:, j * BL:(j + 1) * BL],
                                             AF.Sigmoid, bias=b0_sb[:, j:j + 1])
                    pz2 = ptl.tile([128, 2 * BL], F32, tag="pz2")
                    z2t = cpool.tile([128, 2 * BL], F16, tag="z2t")
                    for m in range(2):
                        for kj in range(4):
                            nc.tensor.matmul(
                                pz2[:, m * BL:(m + 1) * BL],
                                w1_sb[kj][:, m * 128:(m + 1) * 128],
                                z1t[:, kj * BL:(kj + 1) * BL],
                                start=(kj == 0), stop=(kj == 3))
                        nc.scalar.activation(z2t[:, m * BL:(m + 1) * BL],
                                             pz2[:, m * BL:(m + 1) * BL],
                                             AF.Sigmoid, bias=b1_sb[:, m:m + 1])
                    pz3 = ptl.tile([BL, NCLS], F32, tag="pz3")
                    for m in range(2):
                        nc.tensor.matmul(pz3[:], z2t[:, m * BL:(m + 1) * BL],
                                         w2_sb[m][:], start=(m == 0),
                                         stop=(m == 1))
                    z3 = cpool.tile([BL, NCLS], F32, tag="z3")
                    nc.vector.tensor_tensor(z3[:], pz3[:], b2_sb[:], AL.add)
                    mx = cpool.tile([BL, 1], F32, tag="mx")
                    nc.vector.tensor_reduce(mx[:], z3[:], mybir.AxisListType.X,
                                            AL.max)
                    tm = cpool.tile([BL, NCLS], F32, tag="tm")
                    nc.vector.tensor_scalar(tm[:], z3[:], mx[:, 0:1], None,
                                            AL.subtract)
                    ex = cpool.tile([BL, NCLS], F32, tag="ex")
                    se = cpool.tile([BL, 1], F32, tag="se")
                    nc.scalar.activation(ex[:], tm[:], AF.Exp, accum_out=se[:])
                    ls = cpool.tile([BL, 1], F32, tag="ls")
                    nc.scalar.activation(ls[:], se[:], AF.Ln)
                    osb = cpool.tile([BL, NCLS], F32, tag="osb")
                    nc.vector.tensor_scalar(osb[:], tm[:], ls[:, 0:1], None,
                                            AL.subtract)
                    nc.sync.dma_start(out[:], osb[:])
    return nc


_CACHE = {}


def kernel(x, conv_w, conv_b, w0, b0, w1, b1, w2, b2):
    x = np.ascontiguousarray(np.asarray(x, np.float32))
    conv_w = np.asarray(conv_w, np.float32)
    conv_b = np.asarray(conv_b, np.float32)
    w0 = np.asarray(w0, np.float32)
    w1 = np.ascontiguousarray(np.asarray(w1, np.float32))
    w2 = np.ascontiguousarray(np.asarray(w2, np.float32))
    b0 = np.asarray(b0, np.float32)
    b1 = np.asarray(b1, np.float32)
    b2 = np.asarray(b2, np.float32)

    if "nc" not in _CACHE:
        _CACHE["nc"] = _build()
        _CACHE["perm"] = _w0_perm()
        # S3/S4 features arrive /4; S1/S2 natural scale
        sc = np.ones(SIGC, np.float32)
        sc[90:] = FSCALE
        _CACHE["wscale"] = np.tile(sc, OUT_CH)
    nc = _CACHE["nc"]
    w0p = (w0[_CACHE["perm"], :]
           * _CACHE["wscale"][:, None]).astype(np.float16)
    w0pp = np.zeros((NCORES, KSHP, H0), np.float16)
    w0pp[:, :KSH, :] = w0p.reshape(NCORES, KSH, H0)

    shared = {
        "cwr": np.ascontiguousarray(
            np.broadcast_to(conv_w.reshape(1, 16), (128, 16))),
        "cbr": np.ascontiguousarray(
            np.broadcast_to(conv_b.reshape(1, OUT_CH), (128, OUT_CH))),
        "tlin": np.linspace(0.0, 1.0, T, dtype=np.float32).reshape(128, 1),
        "ltri": np.ascontiguousarray(
            np.triu(np.ones((128, 128), np.float32), 1)).astype(np.float16),
        "onec": np.ones((128, 1), np.float16),
        "idh": np.ascontiguousarray(np.eye(64, dtype=np.float16)),
        "onef": np.ones((128, 1), np.float32),
        "idn": np.ascontiguousarray(np.eye(128, BL, dtype=np.float32)),
        "w1s": w1.astype(np.float16), "w2s": w2.astype(np.float16),
        "b0c": b0.reshape(H0, 1), "b1c": b1.reshape(H1, 1),
        "b2r": np.ascontiguousarray(np.broadcast_to(b2.reshape(1, NCLS),
                                                    (BL, NCLS))),
    }
    in_maps = []
    for c in range(NCORES):
        m = dict(shared)
        m["xs"] = np.ascontiguousarray(
            x[c * BL:(c + 1) * BL, 0].transpose(1, 0, 2))
        m["w0s"] = np.ascontiguousarray(w0pp[c])
        in_maps.append(m)

    _CACHE["in_maps"] = in_maps
    res = run_bass_kernel_spmd(nc, in_maps, core_ids=list(range(NCORES)))
    return np.concatenate([res.results[c]["out"] for c in range(NCORES)],
                          axis=0)


# revision 42
# speedup vs baseline: 1.0242x; 1.0140x over previous
"""CNN + truncated path-signature (depth 4) + FF head on 8 TRN2 NeuronCores.

Strategy
--------
- Batch data-parallel signature computation: core c handles batches
  [8c, 8c+8) = 32 (batch, out_ch) lanes, T=128 time steps on partitions.
- Signature reformulated to avoid sequential scans:
    dx, P1 (= shifted path), Y (= suffix sums) are free;
    the only prefix scan (level 2, s2) is one triangular matmul L @ m2;
    S3 = bt^T dx;  S4 = bt^T (dx x Y) + t8^T (dx x dx)/2,
  so levels 3 and 4 are plain T-contractions on the TensorEngine.
- All PE contractions run in fp16 (1 cycle/row; fp32 modes are 4x slower),
  accumulating in fp32 PSUM.  fp16's 10-bit mantissa keeps the final
  rel-err ~2.5e-3 (bf16 operands measured 1.3e-2, too close to the gate).
- w0 (60 MB fp32) is row-sharded 8 ways as fp16 (3.8 MB/core stream):
  AllToAll re-shards the fp16 signature activations feature-wise, each
  core multiplies its [3690, 512] w0 shard for all 64 batches, and a
  fp16 ReduceScatter returns each core its own 8 batches.
- A2A buffer layout is [batch, K] (K fastest) so both the pack DMAs
  (180 B runs) and the post-A2A lhsT gathers (256 B runs) avoid the
  16-byte-run descriptor storm the K-major layout suffers.
- w0 rows are permuted and pre-scaled host-side (S3/S4 features are
  produced /4 on device to keep fp16 headroom), so no on-device
  transposes or rescales are needed.
"""
import os
import sys
sys.path.insert(0, "/opt/trn_rl_repo")
if os.environ.get("JAX_PLATFORMS") == "cpu":
    # The SPMD launch needs the axon/neuron PJRT backend.
    os.environ["JAX_PLATFORMS"] = ""

import numpy as np
import bass_rust as _bass_rust
import concourse.bass as bass
import concourse.tile as tile
import concourse.mybir as mybir
from concourse.vector_clock import ScopedClock
from concourse.bass_utils import run_bass_kernel_spmd

F32 = mybir.dt.float32
F16 = mybir.dt.float16
AL = mybir.AluOpType
AF = mybir.ActivationFunctionType

NCORES = 8
B, T, IN_CH = 64, 128, 32
OUT_CH, CH, D = 4, 8, 9            # conv out-channels, conv width, path dim
BL = B // NCORES                   # local batches = 8
LANES = BL * OUT_CH                # 32 lanes/core
NG = 4                             # lane groups == out-channels
GL = 8                             # lanes per group == local batches
SIGC = 7380                        # per-lane signature channels
KSH = OUT_CH * SIGC // NCORES      # w0 K-shard rows per core = 3690
NCH = 29                           # K chunks of 128
KSHP = NCH * 128                   # shard padded to 3712 (xbar %128 rule)
H0, H1, NCLS = 512, 256, 10
FSCALE = 4.0                       # S3/S4 features arrive /4 (fp16 headroom)
W = LANES * D                      # 288


class _SplitDrainTileContext(tile.TileContext):
    """Tile exit drain carries one sem wait per CTRL instruction.

    This container's walrus build rejects >2 sync waits on a CTRL
    instruction; Tile's exit drain waits on the whole global clock.
    Redistribute the waits over nops on the same engine (program order on
    one engine preserves semantics)."""

    MAX_WAITS = 1

    def _split_body_waits(self):
        """Move excess sem waits from any instruction onto preceding nops on
        the same engine (same-engine program order preserves semantics)."""
        nc = self.nc
        for bb in nc.main_func.blocks:
            heavy = [ins for ins in bb.instructions
                     if ins.sync_info and ins.sync_info.on_wait
                     and len(ins.sync_info.on_wait) > self.MAX_WAITS]
            if not heavy:
                continue
            extra = {}
            for ins in heavy:
                w = list(ins.sync_info.on_wait)
                ins.sync_info.on_wait = w[:self.MAX_WAITS]
                nops = []
                for i in range(self.MAX_WAITS, len(w), self.MAX_WAITS):
                    n = nc.engines[ins.engine].nop(hint="wait_split")
                    # pop the freshly appended nop from wherever it landed
                    for bb2 in nc.main_func.blocks:
                        if bb2.instructions and bb2.instructions[-1] is n.ins:
                            bb2.instructions.pop()
                            break
                    for wt in w[i:i + self.MAX_WAITS]:
                        handle = _bass_rust.SemaphoreHandle(wt.ant_name, wt.id)
                        _bass_rust.wait_op(n.ins, handle, wt.wait_value,
                                           "sem-ge", False)
                    nops.append(n.ins)
                extra[id(ins)] = nops
            new_list = []
            for ins in bb.instructions:
                new_list.extend(extra.get(id(ins), ()))
                new_list.append(ins)
            bb.instructions[:] = new_list

    def _drain_and_barrier(self, tick_clock, wait_clock):
        nc = self.nc
        self._split_body_waits()
        probe = nc.sync.nop(hint="tile_exit_wait_0")
        wait_clock.add_sem_waits(
            probe.ins, ScopedClock({None: tick_clock.global_clock})
        )
        waits = list(probe.ins.sync_info.on_wait or [])
        probe.ins.sync_info.on_wait = waits[:1]
        for w in waits[1:]:
            n = nc.sync.nop(hint="tile_exit_wait")
            handle = _bass_rust.SemaphoreHandle(w.ant_name, w.id)
            _bass_rust.wait_op(n.ins, handle, w.wait_value, "sem-ge", False)
        nc.sync.drain()
        nc.all_engine_barrier()
        assert self.sems is not None
        popped = nc._tile_sem_poison_stack.pop()
        assert popped is self._sem_poison
        nc.clear_and_free_semaphores(list(self.sems.allocated().values()))
        nc.all_engine_barrier()


def _ap(t, extra, *dims):
    """AP over tile t's buffer: partition dim from the tile, custom free dims.

    dims[0] may override the partition [step, count]."""
    base = t[:]
    return bass.AP(base.tensor, base.offset + extra, list(dims))


def _w0_perm():
    """Row permutation p s.t. w0_permuted[i] = w0[p[i]] matches the kernel's
    feature order: per oc: [S1(9) | S2(81)] then rows 90 + ij*90 + c with
    c<81 -> level4 (ij,kl=c), c>=81 -> level3 (ij, k=c-81)."""
    p = np.empty(OUT_CH * SIGC, np.int64)
    i = 0
    for oc in range(OUT_CH):
        base = oc * SIGC
        p[i:i + 90] = base + np.arange(90)          # S1 then S2, native order
        i += 90
        for ij in range(81):
            # level-4 block (ij, kl) for kl in 0..80 -> orig 819 + ij*81 + kl
            p[i:i + 81] = base + 819 + ij * 81 + np.arange(81)
            i += 81
            # level-3 (ij, k) for k in 0..8 -> orig 90 + ij*9 + k
            p[i:i + 9] = base + 90 + ij * 9 + np.arange(9)
            i += 9
    assert i == OUT_CH * SIGC
    return p


def _build():
    nc = bass.Bass(num_devices=NCORES, target_bir_lowering=True, trn_type="TRN2")

    # ---- per-core DRAM inputs ----
    xs = nc.dram_tensor("xs", [T, BL, IN_CH], F32, kind="ExternalInput")
    cwr = nc.dram_tensor("cwr", [128, 16], F32, kind="ExternalInput")
    cbr = nc.dram_tensor("cbr", [128, OUT_CH], F32, kind="ExternalInput")
    tlin = nc.dram_tensor("tlin", [128, 1], F32, kind="ExternalInput")
    ltri = nc.dram_tensor("ltri", [128, 128], F16, kind="ExternalInput")
    onec = nc.dram_tensor("onec", [128, 1], F16, kind="ExternalInput")
    idh = nc.dram_tensor("idh", [64, 64], F16, kind="ExternalInput")
    onef = nc.dram_tensor("onef", [128, 1], F32, kind="ExternalInput")
    idn = nc.dram_tensor("idn", [128, BL], F32, kind="ExternalInput")
    w0s = nc.dram_tensor("w0s", [KSHP, H0], F16, kind="ExternalInput")
    w1s = nc.dram_tensor("w1s", [H0, H1], F16, kind="ExternalInput")
    w2s = nc.dram_tensor("w2s", [H1, NCLS], F16, kind="ExternalInput")
    b0c = nc.dram_tensor("b0c", [H0, 1], F32, kind="ExternalInput")
    b1c = nc.dram_tensor("b1c", [H1, 1], F32, kind="ExternalInput")
    b2r = nc.dram_tensor("b2r", [BL, NCLS], F32, kind="ExternalInput")
    out = nc.dram_tensor("out", [BL, NCLS], F32, kind="ExternalOutput")

    with _SplitDrainTileContext(nc) as tc:
        with tc.tile_pool(name="dram", bufs=1, space="DRAM") as dram:
            # A2A buffers: row r = 8*shard + bloc, K fastest (contiguous)
            zl = dram.tile([B, KSHP], F16)
            zex = dram.tile([B, KSHP], F16)
            cin = dram.tile([B, H0], F32)
            cout = dram.tile([BL, H0], F32)
            prow = dram.tile([1, W], F32)            # p[T-1] bounce

            with tc.tile_pool(name="const", bufs=1) as cpool, \
                 tc.tile_pool(name="w0p", bufs=1) as w0pool, \
                 tc.tile_pool(name="prep", bufs=1) as ppool:
                # ---- const loads ----
                xs_sb = cpool.tile([128, BL * IN_CH], F32)
                nc.scalar.dma_start(xs_sb[:], _ap(xs, 0, [BL * IN_CH, 128],
                                                  [1, BL * IN_CH]))
                cw_sb = cpool.tile([128, 16], F32)
                nc.scalar.dma_start(cw_sb[:], cwr[:])
                cb_sb = cpool.tile([128, OUT_CH], F32)
                nc.scalar.dma_start(cb_sb[:], cbr[:])
                tl_sb = cpool.tile([128, 1], F32)
                nc.scalar.dma_start(tl_sb[:], tlin[:])
                lt_sb = cpool.tile([128, 128], F16)
                nc.scalar.dma_start(lt_sb[:], ltri[:])
                onec_sb = cpool.tile([128, 1], F16)
                nc.scalar.dma_start(onec_sb[:], onec[:])
                idh_sb = cpool.tile([64, 64], F16)
                nc.scalar.dma_start(idh_sb[:], idh[:])
                onef_sb = cpool.tile([128, 1], F32)
                nc.scalar.dma_start(onef_sb[:], onef[:])
                idn_sb = cpool.tile([128, BL], F32)
                nc.scalar.dma_start(idn_sb[:], idn[:])
                w1_sb = [cpool.tile([128, H1], F16, tag=f"w1_{j}", name=f"w1_{j}")
                         for j in range(4)]
                for j in range(4):
                    nc.sync.dma_start(w1_sb[j][:], w1s[j * 128:(j + 1) * 128, :])
                w2_sb = [cpool.tile([128, NCLS], F16, tag=f"w2_{j}", name=f"w2_{j}")
                         for j in range(2)]
                for j in range(2):
                    nc.sync.dma_start(w2_sb[j][:], w2s[j * 128:(j + 1) * 128, :])
                b0_sb = cpool.tile([128, 4], F32)
                nc.scalar.dma_start(b0_sb[:], _ap(b0c, 0, [1, 128], [128, 4]))
                b1_sb = cpool.tile([128, 2], F32)
                nc.scalar.dma_start(b1_sb[:], _ap(b1c, 0, [1, 128], [128, 2]))
                b2_sb = cpool.tile([BL, NCLS], F32)
                nc.scalar.dma_start(b2_sb[:], b2r[:])
                # ---- w0 shard prefetch (streams during signature + A2A) ----
                w0t = [w0pool.tile([128, 14 * H0], F16, tag="w0a", name="w0a"),
                       w0pool.tile([128, 14 * H0], F16, tag="w0b", name="w0b"),
                       w0pool.tile([128, H0], F16, tag="w0c", name="w0c")]
                for h in range(2):
                    nc.sync.dma_start(
                        w0t[h][:],
                        _ap(w0s, h * 14 * 128 * H0, [H0, 128],
                            [128 * H0, 14], [1, H0]))
                nc.sync.dma_start(
                    w0t[2][:],
                    _ap(w0s, 28 * 128 * H0, [H0, 128], [1, H0]))

                # ---- prep: conv -> path p, then dx, P1, Y, ut4, u24, at ----
                # lane order oc-major: lane = oc*8 + bloc  (group g == oc g)
                p = ppool.tile([128, W], F32, tag="p")
                dx = ppool.tile([128, W], F32, tag="dx")
                p1 = ppool.tile([128, W], F32, tag="p1")
                yt = ppool.tile([128, W], F32, tag="yt")
                pl = ppool.tile([128, W], F32, tag="pl")
                at = ppool.tile([128, W], F32, tag="at")
                tmpc = ppool.tile([128, BL * IN_CH], F32, tag="tmpc")
                s1h = ppool.tile([128, W], F16, tag="s1h")

                # conv: p[t, (oc,bloc,c)+1] = sum_k x[t, bloc, 4c+k] w[oc,k]
                pdst = _ap(p, 1, [W, 128], [GL * D, OUT_CH], [D, BL], [1, CH])
                tvw = _ap(tmpc, 0, [BL * IN_CH, 128],
                          [GL * CH, OUT_CH], [CH, BL], [1, CH])

                def xsv(k):
                    return _ap(xs_sb, k, [BL * IN_CH, 128],
                               [0, OUT_CH], [IN_CH, BL], [4, CH])

                def cwv(k):
                    return _ap(cw_sb, k, [16, 128], [4, OUT_CH], [0, BL],
                               [0, CH])

                # split the 4-tap conv across DVE (taps 0,1) / GPSIMD (2,3)
                nc.vector.tensor_tensor(pdst, xsv(0), cwv(0), AL.mult)
                nc.vector.tensor_tensor(tvw, xsv(1), cwv(1), AL.mult)
                nc.vector.tensor_tensor(pdst, pdst, tvw, AL.add)
                tmpg = ppool.tile([128, BL * IN_CH], F32, tag="tmpg")
                tmph = ppool.tile([128, BL * IN_CH], F32, tag="tmph")
                tgw = _ap(tmpg, 0, [BL * IN_CH, 128],
                          [GL * CH, OUT_CH], [CH, BL], [1, CH])
                tgw2 = _ap(tmph, 0, [BL * IN_CH, 128],
                           [GL * CH, OUT_CH], [CH, BL], [1, CH])
                nc.gpsimd.tensor_tensor(tgw, xsv(2), cwv(2), AL.mult)
                nc.gpsimd.tensor_tensor(tgw2, xsv(3), cwv(3), AL.mult)
                nc.gpsimd.tensor_tensor(tgw, tgw, tgw2, AL.add)
                nc.vector.tensor_tensor(pdst, pdst, tgw, AL.add)
                cbv = _ap(cb_sb, 0, [OUT_CH, 128], [1, OUT_CH], [0, BL],
                          [0, CH])
                nc.vector.tensor_tensor(pdst, pdst, cbv, AL.add)
                # time channel into col 0 of every lane
                nc.vector.tensor_copy(_ap(p, 0, [W, 128], [D, LANES]),
                                      _ap(tl_sb, 0, [1, 128], [0, LANES]))
                # P1 = p shifted down one step (DMA: compute engines cannot
                # address unaligned partition bases), then dx = p - P1.
                nc.gpsimd.memset(p1[0:1, :], 0.0)
                nc.gpsimd.dma_start(p1[1:128, :], p[0:127, :])
                nc.vector.tensor_tensor(dx[:], p[:], p1[:], AL.subtract)
                # Y[t] = p[T-1] - p[t]  (broadcast last row via DRAM bounce)
                nc.scalar.dma_start(prow[:], p[127:128, :])
                nc.scalar.dma_start(pl[:], _ap(prow, 0, [0, 128], [1, W]))
                nc.gpsimd.tensor_tensor(yt[:], pl[:], p[:], AL.subtract)
                # S1 row in fp16 (unscaled; w0 S1 rows are x1 host-side);
                # pl has p[T-1] broadcast on every partition -> read row 0
                nc.vector.tensor_copy(s1h[0:1, :], pl[0:1, :])
                # at = P1 + dx/2 (for m2)
                nc.vector.scalar_tensor_tensor(at[:], dx[:], 0.5, p1[:],
                                               AL.mult, AL.add)
                # pre-scaled dx copies (STT is 3D-max; outer products are 4D)
                dr = ppool.tile([128, W], F32, tag="dr")   # dx/4  (rx)
                dq = ppool.tile([128, W], F32, tag="dq")   # dx/8  (q2)
                nc.scalar.activation(dr[:], dx[:], AF.Copy, scale=0.25)
                nc.scalar.activation(dq[:], dx[:], AF.Copy, scale=0.125)

                # ---- per-group signature: g == out-channel ----
                with tc.tile_pool(name="grp", bufs=2) as gpool, \
                     tc.tile_pool(name="ps2", bufs=2, space="PSUM") as ps2p, \
                     tc.tile_pool(name="ptab", bufs=2, space="PSUM") as ptabp:
                    for g in range(NG):
                        off = g * GL * D  # col offset into the 288-wide tiles
                        GW = GL * 81      # 648

                        def o_ij(t, st=1):  # [lane, i(step), j(bcast)] view
                            return _ap(t, off, [W, 128], [D, GL], [st, D],
                                       [0, D])

                        def o_ji(t, st=1):  # [lane, i(bcast), j(step)] view
                            return _ap(t, off, [W, 128], [D, GL], [0, D],
                                       [st, D])

                        # m2[t,(l,ij)] = at_i dx_j  (unscaled, fp16)
                        m2 = gpool.tile([128, GW], F16, tag="m2")
                        m2v = _ap(m2, 0, [GW, 128], [81, GL], [D, D], [1, D])
                        nc.vector.tensor_tensor(m2v, o_ij(at), o_ji(dx),
                                                AL.mult)

                        # s2[t] = sum_{s<t} m2[s]   (fp32 PSUM)
                        s2 = ps2p.tile([128, GW], F32, tag="s2")
                        nc.tensor.matmul(s2[:, 0:512], lt_sb[:], m2[:, 0:512],
                                         start=True, stop=True)
                        nc.tensor.matmul(s2[:, 512:GW], lt_sb[:],
                                         m2[:, 512:GW], start=True, stop=True)
                        # S2 = s2[127] + m2[127]; DVE needs 32-aligned
                        # partition bases, so compute rows 96..127 and let
                        # the header DMA read just row 127
                        z12 = gpool.tile([128, GW], F16, tag="z12")
                        nc.vector.scalar_tensor_tensor(
                            z12[96:128, :], s2[96:128, :], 1.0,
                            m2[96:128, :], AL.mult, AL.add)

                        # q2 = (dx/8) x dx  (= q2_true/4; fp16)
                        q2 = gpool.tile([128, GW], F16, tag="q2")
                        q2v = _ap(q2, 0, [GW, 128], [81, GL], [D, D], [1, D])
                        nc.gpsimd.tensor_tensor(q2v, o_ij(dq), o_ji(dx),
                                                AL.mult)
                        # bt/t8 from m2/q2/s2 algebra (contiguous 2D ops):
                        #   bt = m2/2 - (2/3) q2 + s2 ; t8 = m2/3 - (2/3)q2+s2
                        yc = gpool.tile([128, GW], F16, tag="yc")
                        nc.vector.scalar_tensor_tensor(yc[:], q2[:],
                                                       -2.0 / 3, s2[:],
                                                       AL.mult, AL.add)
                        bt = gpool.tile([128, GW], F16, tag="bt")
                        nc.vector.scalar_tensor_tensor(bt[:], m2[:], 0.5,
                                                       yc[:], AL.mult, AL.add)
                        t8 = gpool.tile([128, GW], F16, tag="t8")
                        nc.vector.scalar_tensor_tensor(t8[:], m2[:], 1.0 / 3,
                                                       yc[:], AL.mult, AL.add)

                        # rx = [(dx/4) x Y | dx/4]  (90 cols per lane)
                        rx = gpool.tile([128, GL * 90], F16, tag="rx")
                        rxv = _ap(rx, 0, [GL * 90, 128], [90, GL], [D, D],
                                  [1, D])
                        nc.vector.tensor_tensor(rxv, o_ij(dr), o_ji(yt),
                                                AL.mult)
                        nc.gpsimd.tensor_copy(
                            _ap(rx, 81, [GL * 90, 128], [90, GL], [1, D]),
                            _ap(dr, off, [W, 128], [D, GL], [1, D]))

                        # tab[l] = bt_l^T rx_l (+) t8_l^T q2_l  -> [81, 90]
                        tab = ptabp.tile([128, 1024], F32, tag="tab")
                        for l in range(GL):
                            nc.tensor.matmul(
                                _ap(tab, 128 * l, [1024, 81], [1, 90]),
                                bt[:, l * 81:(l + 1) * 81],
                                rx[:, l * 90:(l + 1) * 90],
                                start=True, stop=False)
                            nc.tensor.matmul(
                                _ap(tab, 128 * l, [1024, 81], [1, 81]),
                                t8[:, l * 81:(l + 1) * 81],
                                q2[:, l * 81:(l + 1) * 81],
                                start=False, stop=True)
                        # evacuate group: [81, (bloc, 90)] fp16 (ACT engine)
                        zt4 = gpool.tile([81, GL * 90], F16, tag="zt4")
                        nc.scalar.activation(
                            _ap(zt4, 0, [GL * 90, 81], [90, GL], [1, 90]),
                            _ap(tab, 0, [1024, 81], [128, GL], [1, 90]),
                            AF.Copy)

                        # ---- pack this group's features into zl ----
                        # lane (oc=g, bloc) shard 2g:   rows 16g + bloc
                        #                   shard 2g+1: rows 16g + 8 + bloc
                        e1 = nc.sync if g % 2 == 0 else nc.scalar
                        e2 = nc.scalar if g % 2 == 0 else nc.sync
                        # S2 header (81 els at col 9)
                        e1.dma_start(
                            _ap(zl, (16 * g) * KSHP + 9, [KSHP, GL], [1, 81]),
                            z12[127:128, :])
                        # ij 0..39 -> shard 2g cols [90 + ij*90 + kl]
                        e1.dma_start(
                            _ap(zl, (16 * g) * KSHP + 90, [90, 40],
                                [KSHP, GL], [1, 90]),
                            _ap(zt4, 0, [GL * 90, 40], [90, GL], [1, 90]))
                        # ij 40..80 -> shard 2g+1 cols [(ij-40)*90 + kl]
                        e2.dma_start(
                            _ap(zl, (16 * g + 8) * KSHP, [90, 41],
                                [KSHP, GL], [1, 90]),
                            _ap(zt4, 40 * GL * 90, [GL * 90, 41],
                                [90, GL], [1, 90]))

                    # S1 headers for all lanes (9 els at col 0); s1h row 0
                    # natural col order (oc, bloc, d) matches the dst order
                    nc.sync.dma_start(
                        _ap(zl, 0, [16 * KSHP, OUT_CH], [KSHP, GL], [1, D]),
                        s1h[0:1, :])
                    # zero the 22-el xbar pad of every row
                    zpad = ppool.tile([1, B * (KSHP - KSH)], F16, tag="zpad")
                    nc.gpsimd.memset(zpad[0:1, :], 0.0)
                    nc.scalar.dma_start(
                        _ap(zl, KSH, [KSHP, B], [1, KSHP - KSH]),
                        zpad[0:1, :])

                    nc.gpsimd.collective_compute(
                        "AllToAll", AL.bypass,
                        replica_groups=[list(range(NCORES))],
                        ins=[zl[:].opt()], outs=[zex[:].opt()])

                # ---- z0 = z @ w0 partial over this core's K shard ----
                with tc.tile_pool(name="zt", bufs=1) as ztp, \
                     tc.tile_pool(name="ptr", bufs=2, space="PSUM") as ptrp, \
                     tc.tile_pool(name="pz0", bufs=1, space="PSUM") as pz0p, \
                     tc.tile_pool(name="ptail", bufs=1, space="PSUM") as ptl:
                    # one fully-contiguous batch-major gather (7.4 KB runs),
                    # then PE-transpose each [64, 128] block to [128 K, 64 b]
                    zb = ztp.tile([B, KSHP], F16, tag="zb", name="zb")
                    nc.sync.dma_start(zb[:], zex[:])
                    zT = [ztp.tile([128, 14 * B], F16, tag="zta", name="zta"),
                          ztp.tile([128, 14 * B], F16, tag="ztb", name="ztb"),
                          ztp.tile([128, B], F16, tag="ztc", name="ztc")]

                    def zt_chunk(i):
                        if i < 28:
                            return zT[i // 14][:, (i % 14) * B:
                                               (i % 14 + 1) * B]
                        return zT[2][:]

                    # all transposes first (HAM ignores transpose-mode, so
                    # interleaving would keep the PE cold), then a dense
                    # matmul burst that warms up and stays warm
                    for i in range(NCH):
                        ptr = ptrp.tile([128, B], F16, tag="ptr")
                        nc.tensor.transpose(ptr[:],
                                            zb[:, i * 128:(i + 1) * 128],
                                            idh_sb[:])
                        if i % 2 == 0:
                            nc.vector.tensor_copy(zt_chunk(i), ptr[:])
                        else:
                            nc.scalar.activation(zt_chunk(i), ptr[:], AF.Copy)
                    z0p = pz0p.tile([B, H0], F32, tag="z0p")
                    for i in range(NCH):
                        if i < 28:
                            rhs = w0t[i // 14][:, (i % 14) * H0:
                                               (i % 14 + 1) * H0]
                        else:
                            rhs = w0t[2][:]
                        nc.tensor.matmul(z0p[:], zt_chunk(i), rhs,
                                         start=(i == 0), stop=(i == NCH - 1))
                    z0sb = cpool.tile([B, H0], F32, tag="z0sb", name="z0sb")
                    nc.vector.tensor_copy(z0sb[:], z0p[:])
                    nc.sync.dma_start(cin[:], z0sb[:])
                    nc.gpsimd.collective_compute(
                        "ReduceScatter", AL.add,
                        replica_groups=[list(range(NCORES))],
                        ins=[cin[:].opt()], outs=[cout[:].opt()])

                    # preload ACT tables during the RS window (dead time,
                    # and after the last AF.Copy use so nothing re-evicts)
                    dum = ppool.tile([1, 4], F32, tag="dum")
                    nc.scalar.activation(dum[0:1, 0:1], tl_sb[0:1, 0:1],
                                         AF.Sigmoid)
                    nc.scalar.activation(dum[0:1, 1:2], tl_sb[0:1, 0:1],
                                         AF.Exp)
                    nc.scalar.activation(dum[0:1, 2:3], onef_sb[0:1, 0:1],
                                         AF.Ln)

                    # ---- tail: sigmoid(z0) -> w1 -> sigmoid -> w2 ----
                    z1row = cpool.tile([BL, H0], F32, tag="z1row")
                    nc.sync.dma_start(z1row[:], cout[:])
                    pz1 = ptl.tile([128, 4 * BL], F32, tag="pz1")
                    z1t = cpool.tile([128, 4 * BL], F16, tag="z1t")
                    for j in range(4):
                        nc.tensor.transpose(pz1[:, j * BL:(j + 1) * BL],
                                            z1row[:, j * 128:(j + 1) * 128],
                                            idn_sb[0:BL, 0:BL])
                        nc.scalar.activation(z1t[:, j * BL:(j + 1) * BL],
                                             pz1[:, j * BL:(j + 1) * BL],
                                             AF.Sigmoid, bias=b0_sb[:, j:j + 1])
                    pz2 = ptl.tile([128, 2 * BL], F32, tag="pz2")
                    z2t = cpool.tile([128, 2 * BL], F16, tag="z2t")
                    for m in range(2):
                        for kj in range(4):
                            nc.tensor.matmul(
                                pz2[:, m * BL:(m + 1) * BL],
                                w1_sb[kj][:, m * 128:(m + 1) * 128],
                                z1t[:, kj * BL:(kj + 1) * BL],
                                start=(kj == 0), stop=(kj == 3))
                        nc.scalar.activation(z2t[:, m * BL:(m + 1) * BL],
                                             pz2[:, m * BL:(m + 1) * BL],
                                             AF.Sigmoid, bias=b1_sb[:, m:m + 1])
                    pz3 = ptl.tile([BL, NCLS], F32, tag="pz3")
                    for m in range(2):
                        nc.tensor.matmul(pz3[:], z2t[:, m * BL:(m + 1) * BL],
                                         w2_sb[m][:], start=(m == 0),
                                         stop=(m == 1))
                    z3 = cpool.tile([BL, NCLS], F32, tag="z3")
                    nc.vector.tensor_tensor(z3[:], pz3[:], b2_sb[:], AL.add)
                    mx = cpool.tile([BL, 1], F32, tag="mx")
                    nc.vector.tensor_reduce(mx[:], z3[:], mybir.AxisListType.X,
                                            AL.max)
                    tm = cpool.tile([BL, NCLS], F32, tag="tm")
                    nc.vector.tensor_scalar(tm[:], z3[:], mx[:, 0:1], None,
                                            AL.subtract)
                    ex = cpool.tile([BL, NCLS], F32, tag="ex")
                    se = cpool.tile([BL, 1], F32, tag="se")
                    nc.scalar.activation(ex[:], tm[:], AF.Exp, accum_out=se[:])
                    ls = cpool.tile([BL, 1], F32, tag="ls")
                    nc.scalar.activation(ls[:], se[:], AF.Ln)
                    osb = cpool.tile([BL, NCLS], F32, tag="osb")
                    nc.vector.tensor_scalar(osb[:], tm[:], ls[:, 0:1], None,
                                            AL.subtract)
                    nc.sync.dma_start(out[:], osb[:])
    return nc


_CACHE = {}


def kernel(x, conv_w, conv_b, w0, b0, w1, b1, w2, b2):
    x = np.ascontiguousarray(np.asarray(x, np.float32))
    conv_w = np.asarray(conv_w, np.float32)
    conv_b = np.asarray(conv_b, np.float32)
    w0 = np.asarray(w0, np.float32)
    w1 = np.ascontiguousarray(np.asarray(w1, np.float32))
    w2 = np.ascontiguousarray(np.asarray(w2, np.float32))
    b0 = np.asarray(b0, np.float32)
    b1 = np.asarray(b1, np.float32)
    b2 = np.asarray(b2, np.float32)

    if "nc" not in _CACHE:
        _CACHE["nc"] = _build()
        _CACHE["perm"] = _w0_perm()
        # S3/S4 features arrive /4; S1/S2 natural scale
        sc = np.ones(SIGC, np.float32)
        sc[90:] = FSCALE
        _CACHE["wscale"] = np.tile(sc, OUT_CH)
    nc = _CACHE["nc"]
    w0p = (w0[_CACHE["perm"], :]
           * _CACHE["wscale"][:, None]).astype(np.float16)
    w0pp = np.zeros((NCORES, KSHP, H0), np.float16)
    w0pp[:, :KSH, :] = w0p.reshape(NCORES, KSH, H0)

    shared = {
        "cwr": np.ascontiguousarray(
            np.broadcast_to(conv_w.reshape(1, 16), (128, 16))),
        "cbr": np.ascontiguousarray(
            np.broadcast_to(conv_b.reshape(1, OUT_CH), (128, OUT_CH))),
        "tlin": np.linspace(0.0, 1.0, T, dtype=np.float32).reshape(128, 1),
        "ltri": np.ascontiguousarray(
            np.triu(np.ones((128, 128), np.float32), 1)).astype(np.float16),
        "onec": np.ones((128, 1), np.float16),
        "idh": np.ascontiguousarray(np.eye(64, dtype=np.float16)),
        "onef": np.ones((128, 1), np.float32),
        "idn": np.ascontiguousarray(np.eye(128, BL, dtype=np.float32)),
        "w1s": w1.astype(np.float16), "w2s": w2.astype(np.float16),
        "b0c": b0.reshape(H0, 1), "b1c": b1.reshape(H1, 1),
        "b2r": np.ascontiguousarray(np.broadcast_to(b2.reshape(1, NCLS),
                                                    (BL, NCLS))),
    }
    in_maps = []
    for c in range(NCORES):
        m = dict(shared)
        m["xs"] = np.ascontiguousarray(
            x[c * BL:(c + 1) * BL, 0].transpose(1, 0, 2))
        m["w0s"] = np.ascontiguousarray(w0pp[c])
        in_maps.append(m)

    _CACHE["in_maps"] = in_maps
    res = run_bass_kernel_spmd(nc, in_maps, core_ids=list(range(NCORES)))
    return np.concatenate([res.results[c]["out"] for c in range(NCORES)],
                          axis=0)


# revision 44
# speedup vs baseline: 1.0277x; 1.0035x over previous
"""CNN + truncated path-signature (depth 4) + FF head on 8 TRN2 NeuronCores.

Strategy
--------
- Batch data-parallel signature computation: core c handles batches
  [8c, 8c+8) = 32 (batch, out_ch) lanes, T=128 time steps on partitions.
- Signature reformulated to avoid sequential scans:
    dx, P1 (= shifted path), Y (= suffix sums) are free;
    the only prefix scan (level 2, s2) is one triangular matmul L @ m2;
    S3 = bt^T dx;  S4 = bt^T (dx x Y) + t8^T (dx x dx)/2,
  so levels 3 and 4 are plain T-contractions on the TensorEngine.
- All PE contractions run in fp16 (1 cycle/row; fp32 modes are 4x slower),
  accumulating in fp32 PSUM.  fp16's 10-bit mantissa keeps the final
  rel-err ~2.5e-3 (bf16 operands measured 1.3e-2, too close to the gate).
- w0 (60 MB fp32) is row-sharded 8 ways as fp16 (3.8 MB/core stream):
  AllToAll re-shards the fp16 signature activations feature-wise, each
  core multiplies its [3690, 512] w0 shard for all 64 batches, and an
  fp32 ReduceScatter returns each core its own 8 batches (partials are
  ~100x larger than the final logits, so 16-bit RS is NOT safe).
- A2A buffer layout is [batch, K] (K fastest): pack DMAs write 180 B
  runs and the post-A2A gather is one fully-contiguous 7.4 KB-run DMA,
  avoiding the 16-byte-run descriptor storm of a K-major layout.  The
  [64, 128] -> [128, 64] chunk transposes run on the PE (the xbar
  DMA-transpose path has broken completion accounting here and races
  with its consumers).  All transposes are issued before the matmul
  burst because HAM ignores transpose-mode and would keep the PE cold.
- bt/t8 are reconstructed as c*m2 - (2/3) q2 + s2 (one shared stt
  term), replacing four strided outer-product ops per group with one
  PSUM op and two fast contiguous ones.
- w0 rows are permuted and pre-scaled host-side (S3/S4 features are
  produced /4 on device to keep fp16 headroom), so no on-device
  transposes or rescales are needed.
"""
import os
import sys
sys.path.insert(0, "/opt/trn_rl_repo")
if os.environ.get("JAX_PLATFORMS") == "cpu":
    # The SPMD launch needs the axon/neuron PJRT backend.
    os.environ["JAX_PLATFORMS"] = ""

import numpy as np
import bass_rust as _bass_rust
import concourse.bass as bass
import concourse.tile as tile
import concourse.mybir as mybir
from concourse.vector_clock import ScopedClock
from concourse.bass_utils import run_bass_kernel_spmd

F32 = mybir.dt.float32
F16 = mybir.dt.float16
AL = mybir.AluOpType
AF = mybir.ActivationFunctionType

NCORES = 8
B, T, IN_CH = 64, 128, 32
OUT_CH, CH, D = 4, 8, 9            # conv out-channels, conv width, path dim
BL = B // NCORES                   # local batches = 8
LANES = BL * OUT_CH                # 32 lanes/core
NG = 4                             # lane groups == out-channels
GL = 8                             # lanes per group == local batches
SIGC = 7380                        # per-lane signature channels
KSH = OUT_CH * SIGC // NCORES      # w0 K-shard rows per core = 3690
NCH = 29                           # K chunks of 128
KSHP = NCH * 128                   # shard padded to 3712 (xbar %128 rule)
H0, H1, NCLS = 512, 256, 10
FSCALE = 4.0                       # S3/S4 features arrive /4 (fp16 headroom)
W = LANES * D                      # 288


class _SplitDrainTileContext(tile.TileContext):
    """Tile exit drain carries one sem wait per CTRL instruction.

    This container's walrus build rejects >2 sync waits on a CTRL
    instruction; Tile's exit drain waits on the whole global clock.
    Redistribute the waits over nops on the same engine (program order on
    one engine preserves semantics)."""

    MAX_WAITS = 1

    def _split_body_waits(self):
        """Move excess sem waits from any instruction onto preceding nops on
        the same engine (same-engine program order preserves semantics)."""
        nc = self.nc
        for bb in nc.main_func.blocks:
            heavy = [ins for ins in bb.instructions
                     if ins.sync_info and ins.sync_info.on_wait
                     and len(ins.sync_info.on_wait) > self.MAX_WAITS]
            if not heavy:
                continue
            extra = {}
            for ins in heavy:
                w = list(ins.sync_info.on_wait)
                ins.sync_info.on_wait = w[:self.MAX_WAITS]
                nops = []
                for i in range(self.MAX_WAITS, len(w), self.MAX_WAITS):
                    n = nc.engines[ins.engine].nop(hint="wait_split")
                    # pop the freshly appended nop from wherever it landed
                    for bb2 in nc.main_func.blocks:
                        if bb2.instructions and bb2.instructions[-1] is n.ins:
                            bb2.instructions.pop()
                            break
                    for wt in w[i:i + self.MAX_WAITS]:
                        handle = _bass_rust.SemaphoreHandle(wt.ant_name, wt.id)
                        _bass_rust.wait_op(n.ins, handle, wt.wait_value,
                                           "sem-ge", False)
                    nops.append(n.ins)
                extra[id(ins)] = nops
            new_list = []
            for ins in bb.instructions:
                new_list.extend(extra.get(id(ins), ()))
                new_list.append(ins)
            bb.instructions[:] = new_list

    def _drain_and_barrier(self, tick_clock, wait_clock):
        nc = self.nc
        self._split_body_waits()
        probe = nc.sync.nop(hint="tile_exit_wait_0")
        wait_clock.add_sem_waits(
            probe.ins, ScopedClock({None: tick_clock.global_clock})
        )
        waits = list(probe.ins.sync_info.on_wait or [])
        probe.ins.sync_info.on_wait = waits[:1]
        for w in waits[1:]:
            n = nc.sync.nop(hint="tile_exit_wait")
            handle = _bass_rust.SemaphoreHandle(w.ant_name, w.id)
            _bass_rust.wait_op(n.ins, handle, w.wait_value, "sem-ge", False)
        nc.sync.drain()
        nc.all_engine_barrier()
        assert self.sems is not None
        popped = nc._tile_sem_poison_stack.pop()
        assert popped is self._sem_poison
        nc.clear_and_free_semaphores(list(self.sems.allocated().values()))
        nc.all_engine_barrier()


def _ap(t, extra, *dims):
    """AP over tile t's buffer: partition dim from the tile, custom free dims.

    dims[0] may override the partition [step, count]."""
    base = t[:]
    return bass.AP(base.tensor, base.offset + extra, list(dims))


def _w0_perm():
    """Row permutation p s.t. w0_permuted[i] = w0[p[i]] matches the kernel's
    feature order: per oc: [S1(9) | S2(81)] then rows 90 + ij*90 + c with
    c<81 -> level4 (ij,kl=c), c>=81 -> level3 (ij, k=c-81)."""
    p = np.empty(OUT_CH * SIGC, np.int64)
    i = 0
    for oc in range(OUT_CH):
        base = oc * SIGC
        p[i:i + 90] = base + np.arange(90)          # S1 then S2, native order
        i += 90
        for ij in range(81):
            # level-4 block (ij, kl) for kl in 0..80 -> orig 819 + ij*81 + kl
            p[i:i + 81] = base + 819 + ij * 81 + np.arange(81)
            i += 81
            # level-3 (ij, k) for k in 0..8 -> orig 90 + ij*9 + k
            p[i:i + 9] = base + 90 + ij * 9 + np.arange(9)
            i += 9
    assert i == OUT_CH * SIGC
    return p


def _build():
    nc = bass.Bass(num_devices=NCORES, target_bir_lowering=True, trn_type="TRN2")

    # ---- per-core DRAM inputs ----
    xs = nc.dram_tensor("xs", [T, BL, IN_CH], F32, kind="ExternalInput")
    cwr = nc.dram_tensor("cwr", [128, 16], F32, kind="ExternalInput")
    cbr = nc.dram_tensor("cbr", [128, OUT_CH], F32, kind="ExternalInput")
    tlin = nc.dram_tensor("tlin", [128, 1], F32, kind="ExternalInput")
    ltri = nc.dram_tensor("ltri", [128, 128], F16, kind="ExternalInput")
    onec = nc.dram_tensor("onec", [128, 1], F16, kind="ExternalInput")
    idh = nc.dram_tensor("idh", [64, 64], F16, kind="ExternalInput")
    onef = nc.dram_tensor("onef", [128, 1], F32, kind="ExternalInput")
    idn = nc.dram_tensor("idn", [128, BL], F32, kind="ExternalInput")
    w0s = nc.dram_tensor("w0s", [KSHP, H0], F16, kind="ExternalInput")
    w1s = nc.dram_tensor("w1s", [H0, H1], F16, kind="ExternalInput")
    w2s = nc.dram_tensor("w2s", [H1, NCLS], F16, kind="ExternalInput")
    b0c = nc.dram_tensor("b0c", [H0, 1], F32, kind="ExternalInput")
    b1c = nc.dram_tensor("b1c", [H1, 1], F32, kind="ExternalInput")
    b2r = nc.dram_tensor("b2r", [BL, NCLS], F32, kind="ExternalInput")
    out = nc.dram_tensor("out", [BL, NCLS], F32, kind="ExternalOutput")

    with _SplitDrainTileContext(nc) as tc:
        with tc.tile_pool(name="dram", bufs=1, space="DRAM") as dram:
            # A2A buffers: row r = 8*shard + bloc, K fastest (contiguous)
            zl = dram.tile([B, KSHP], F16)
            zex = dram.tile([B, KSHP], F16)
            cin = dram.tile([B, H0], F32)
            cout = dram.tile([BL, H0], F32)
            prow = dram.tile([1, W], F32)            # p[T-1] bounce

            with tc.tile_pool(name="const", bufs=1) as cpool, \
                 tc.tile_pool(name="w0p", bufs=1) as w0pool, \
                 tc.tile_pool(name="prep", bufs=1) as ppool:
                # ---- const loads ----
                xs_sb = cpool.tile([128, BL * IN_CH], F32)
                nc.scalar.dma_start(xs_sb[:], _ap(xs, 0, [BL * IN_CH, 128],
                                                  [1, BL * IN_CH]))
                cw_sb = cpool.tile([128, 16], F32)
                nc.scalar.dma_start(cw_sb[:], cwr[:])
                cb_sb = cpool.tile([128, OUT_CH], F32)
                nc.scalar.dma_start(cb_sb[:], cbr[:])
                tl_sb = cpool.tile([128, 1], F32)
                nc.scalar.dma_start(tl_sb[:], tlin[:])
                lt_sb = cpool.tile([128, 128], F16)
                nc.scalar.dma_start(lt_sb[:], ltri[:])
                onec_sb = cpool.tile([128, 1], F16)
                nc.scalar.dma_start(onec_sb[:], onec[:])
                idh_sb = cpool.tile([64, 64], F16)
                nc.scalar.dma_start(idh_sb[:], idh[:])
                onef_sb = cpool.tile([128, 1], F32)
                nc.scalar.dma_start(onef_sb[:], onef[:])
                idn_sb = cpool.tile([128, BL], F32)
                nc.scalar.dma_start(idn_sb[:], idn[:])
                w1_sb = [cpool.tile([128, H1], F16, tag=f"w1_{j}", name=f"w1_{j}")
                         for j in range(4)]
                for j in range(4):
                    nc.sync.dma_start(w1_sb[j][:], w1s[j * 128:(j + 1) * 128, :])
                w2_sb = [cpool.tile([128, NCLS], F16, tag=f"w2_{j}", name=f"w2_{j}")
                         for j in range(2)]
                for j in range(2):
                    nc.sync.dma_start(w2_sb[j][:], w2s[j * 128:(j + 1) * 128, :])
                b0_sb = cpool.tile([128, 4], F32)
                nc.scalar.dma_start(b0_sb[:], _ap(b0c, 0, [1, 128], [128, 4]))
                b1_sb = cpool.tile([128, 2], F32)
                nc.scalar.dma_start(b1_sb[:], _ap(b1c, 0, [1, 128], [128, 2]))
                b2_sb = cpool.tile([BL, NCLS], F32)
                nc.scalar.dma_start(b2_sb[:], b2r[:])
                # ---- prep: conv -> path p, then dx, P1, Y, ut4, u24, at ----
                # lane order oc-major: lane = oc*8 + bloc  (group g == oc g)
                p = ppool.tile([128, W], F32, tag="p")
                dx = ppool.tile([128, W], F32, tag="dx")
                p1 = ppool.tile([128, W], F32, tag="p1")
                yt = ppool.tile([128, W], F32, tag="yt")
                pl = ppool.tile([128, W], F32, tag="pl")
                at = ppool.tile([128, W], F32, tag="at")
                tmpc = ppool.tile([128, BL * IN_CH], F32, tag="tmpc")
                s1h = ppool.tile([128, W], F16, tag="s1h")

                # conv: p[t, (oc,bloc,c)+1] = sum_k x[t, bloc, 4c+k] w[oc,k]
                pdst = _ap(p, 1, [W, 128], [GL * D, OUT_CH], [D, BL], [1, CH])
                tvw = _ap(tmpc, 0, [BL * IN_CH, 128],
                          [GL * CH, OUT_CH], [CH, BL], [1, CH])

                def xsv(k):
                    return _ap(xs_sb, k, [BL * IN_CH, 128],
                               [0, OUT_CH], [IN_CH, BL], [4, CH])

                def cwv(k):
                    return _ap(cw_sb, k, [16, 128], [4, OUT_CH], [0, BL],
                               [0, CH])

                # split the 4-tap conv across DVE (taps 0,1) / GPSIMD (2,3)
                nc.vector.tensor_tensor(pdst, xsv(0), cwv(0), AL.mult)
                nc.vector.tensor_tensor(tvw, xsv(1), cwv(1), AL.mult)
                nc.vector.tensor_tensor(pdst, pdst, tvw, AL.add)
                tmpg = ppool.tile([128, BL * IN_CH], F32, tag="tmpg")
                tmph = ppool.tile([128, BL * IN_CH], F32, tag="tmph")
                tgw = _ap(tmpg, 0, [BL * IN_CH, 128],
                          [GL * CH, OUT_CH], [CH, BL], [1, CH])
                tgw2 = _ap(tmph, 0, [BL * IN_CH, 128],
                           [GL * CH, OUT_CH], [CH, BL], [1, CH])
                nc.gpsimd.tensor_tensor(tgw, xsv(2), cwv(2), AL.mult)
                nc.gpsimd.tensor_tensor(tgw2, xsv(3), cwv(3), AL.mult)
                nc.gpsimd.tensor_tensor(tgw, tgw, tgw2, AL.add)
                nc.vector.tensor_tensor(pdst, pdst, tgw, AL.add)
                cbv = _ap(cb_sb, 0, [OUT_CH, 128], [1, OUT_CH], [0, BL],
                          [0, CH])
                nc.vector.tensor_tensor(pdst, pdst, cbv, AL.add)
                # time channel into col 0 of every lane
                nc.vector.tensor_copy(_ap(p, 0, [W, 128], [D, LANES]),
                                      _ap(tl_sb, 0, [1, 128], [0, LANES]))
                # P1 = p shifted down one step (DMA: compute engines cannot
                # address unaligned partition bases), then dx = p - P1.
                nc.gpsimd.memset(p1[0:1, :], 0.0)
                nc.gpsimd.dma_start(p1[1:128, :], p[0:127, :])
                nc.vector.tensor_tensor(dx[:], p[:], p1[:], AL.subtract)
                # Y[t] = p[T-1] - p[t]  (broadcast last row via DRAM bounce)
                nc.scalar.dma_start(prow[:], p[127:128, :])
                nc.scalar.dma_start(pl[:], _ap(prow, 0, [0, 128], [1, W]))
                nc.gpsimd.tensor_tensor(yt[:], pl[:], p[:], AL.subtract)
                # S1 row in fp16 (unscaled; w0 S1 rows are x1 host-side);
                # pl has p[T-1] broadcast on every partition -> read row 0
                nc.vector.tensor_copy(s1h[0:1, :], pl[0:1, :])
                # at = P1 + dx/2 (for m2)
                nc.vector.scalar_tensor_tensor(at[:], dx[:], 0.5, p1[:],
                                               AL.mult, AL.add)
                # pre-scaled dx copies (STT is 3D-max; outer products are 4D)
                dr = ppool.tile([128, W], F32, tag="dr")   # dx/4  (rx)
                dq = ppool.tile([128, W], F32, tag="dq")   # dx/8  (q2)
                nc.scalar.activation(dr[:], dx[:], AF.Copy, scale=0.25)
                nc.scalar.activation(dq[:], dx[:], AF.Copy, scale=0.125)

                # ---- w0 shard prefetch (after prep DMAs so they get
                # queue priority; streams during signature + A2A) ----
                w0t = [w0pool.tile([128, 14 * H0], F16, tag="w0a", name="w0a"),
                       w0pool.tile([128, 14 * H0], F16, tag="w0b", name="w0b"),
                       w0pool.tile([128, H0], F16, tag="w0c", name="w0c")]
                for h in range(2):
                    nc.sync.dma_start(
                        w0t[h][:],
                        _ap(w0s, h * 14 * 128 * H0, [H0, 128],
                            [128 * H0, 14], [1, H0]))
                nc.sync.dma_start(
                    w0t[2][:],
                    _ap(w0s, 28 * 128 * H0, [H0, 128], [1, H0]))
                # zero the 22-el xbar pad of every zl row (independent)
                zpad = ppool.tile([1, B * (KSHP - KSH)], F16, tag="zpad")
                nc.gpsimd.memset(zpad[0:1, :], 0.0)
                nc.scalar.dma_start(
                    _ap(zl, KSH, [KSHP, B], [1, KSHP - KSH]),
                    zpad[0:1, :])

                # ---- per-group signature: g == out-channel ----
                with tc.tile_pool(name="grp", bufs=2) as gpool, \
                     tc.tile_pool(name="ps2", bufs=2, space="PSUM") as ps2p, \
                     tc.tile_pool(name="ptab", bufs=2, space="PSUM") as ptabp:
                    for g in range(NG):
                        off = g * GL * D  # col offset into the 288-wide tiles
                        GW = GL * 81      # 648

                        def o_ij(t, st=1):  # [lane, i(step), j(bcast)] view
                            return _ap(t, off, [W, 128], [D, GL], [st, D],
                                       [0, D])

                        def o_ji(t, st=1):  # [lane, i(bcast), j(step)] view
                            return _ap(t, off, [W, 128], [D, GL], [0, D],
                                       [st, D])

                        # m2[t,(l,ij)] = at_i dx_j  (unscaled, fp16)
                        m2 = gpool.tile([128, GW], F16, tag="m2")
                        m2v = _ap(m2, 0, [GW, 128], [81, GL], [D, D], [1, D])
                        nc.vector.tensor_tensor(m2v, o_ij(at), o_ji(dx),
                                                AL.mult)

                        # s2[t] = sum_{s<t} m2[s]   (fp32 PSUM)
                        s2 = ps2p.tile([128, GW], F32, tag="s2")
                        nc.tensor.matmul(s2[:, 0:512], lt_sb[:], m2[:, 0:512],
                                         start=True, stop=True)
                        nc.tensor.matmul(s2[:, 512:GW], lt_sb[:],
                                         m2[:, 512:GW], start=True, stop=True)
                        # S2 = s2[127] + m2[127]; DVE needs 32-aligned
                        # partition bases, so compute rows 96..127 and let
                        # the header DMA read just row 127
                        z12 = gpool.tile([128, GW], F16, tag="z12")
                        nc.vector.scalar_tensor_tensor(
                            z12[96:128, :], s2[96:128, :], 1.0,
                            m2[96:128, :], AL.mult, AL.add)

                        # q2 = (dx/8) x dx  (= q2_true/4; fp16)
                        q2 = gpool.tile([128, GW], F16, tag="q2")
                        q2v = _ap(q2, 0, [GW, 128], [81, GL], [D, D], [1, D])
                        nc.gpsimd.tensor_tensor(q2v, o_ij(dq), o_ji(dx),
                                                AL.mult)
                        # bt/t8 from m2/q2/s2 algebra (contiguous 2D ops):
                        #   bt = m2/2 - (2/3) q2 + s2 ; t8 = m2/3 - (2/3)q2+s2
                        yc = gpool.tile([128, GW], F16, tag="yc")
                        nc.vector.scalar_tensor_tensor(yc[:], q2[:],
                                                       -2.0 / 3, s2[:],
                                                       AL.mult, AL.add)
                        bt = gpool.tile([128, GW], F16, tag="bt")
                        nc.vector.scalar_tensor_tensor(bt[:], m2[:], 0.5,
                                                       yc[:], AL.mult, AL.add)
                        t8 = gpool.tile([128, GW], F16, tag="t8")
                        nc.vector.scalar_tensor_tensor(t8[:], m2[:], 1.0 / 3,
                                                       yc[:], AL.mult, AL.add)

                        # rx = [(dx/4) x Y | dx/4]  (90 cols per lane)
                        rx = gpool.tile([128, GL * 90], F16, tag="rx")
                        rxv = _ap(rx, 0, [GL * 90, 128], [90, GL], [D, D],
                                  [1, D])
                        nc.vector.tensor_tensor(rxv, o_ij(dr), o_ji(yt),
                                                AL.mult)
                        nc.gpsimd.tensor_copy(
                            _ap(rx, 81, [GL * 90, 128], [90, GL], [1, D]),
                            _ap(dr, off, [W, 128], [D, GL], [1, D]))

                        # tab[l] = bt_l^T rx_l (+) t8_l^T q2_l  -> [81, 90]
                        tab = ptabp.tile([128, 1024], F32, tag="tab")
                        for l in range(GL):
                            nc.tensor.matmul(
                                _ap(tab, 128 * l, [1024, 81], [1, 90]),
                                bt[:, l * 81:(l + 1) * 81],
                                rx[:, l * 90:(l + 1) * 90],
                                start=True, stop=False)
                            nc.tensor.matmul(
                                _ap(tab, 128 * l, [1024, 81], [1, 81]),
                                t8[:, l * 81:(l + 1) * 81],
                                q2[:, l * 81:(l + 1) * 81],
                                start=False, stop=True)
                        # evacuate group: [81, (bloc, 90)] fp16 (ACT engine)
                        zt4 = gpool.tile([81, GL * 90], F16, tag="zt4")
                        nc.scalar.activation(
                            _ap(zt4, 0, [GL * 90, 81], [90, GL], [1, 90]),
                            _ap(tab, 0, [1024, 81], [128, GL], [1, 90]),
                            AF.Copy)

                        # ---- pack this group's features into zl ----
                        # lane (oc=g, bloc) shard 2g:   rows 16g + bloc
                        #                   shard 2g+1: rows 16g + 8 + bloc
                        e1 = nc.sync if g % 2 == 0 else nc.scalar
                        e2 = nc.scalar if g % 2 == 0 else nc.sync
                        # S2 header (81 els at col 9)
                        e1.dma_start(
                            _ap(zl, (16 * g) * KSHP + 9, [KSHP, GL], [1, 81]),
                            z12[127:128, :])
                        # ij 0..39 -> shard 2g cols [90 + ij*90 + kl]
                        e1.dma_start(
                            _ap(zl, (16 * g) * KSHP + 90, [90, 40],
                                [KSHP, GL], [1, 90]),
                            _ap(zt4, 0, [GL * 90, 40], [90, GL], [1, 90]))
                        # ij 40..80 -> shard 2g+1 cols [(ij-40)*90 + kl]
                        e2.dma_start(
                            _ap(zl, (16 * g + 8) * KSHP, [90, 41],
                                [KSHP, GL], [1, 90]),
                            _ap(zt4, 40 * GL * 90, [GL * 90, 41],
                                [90, GL], [1, 90]))

                    # S1 headers for all lanes (9 els at col 0); s1h row 0
                    # natural col order (oc, bloc, d) matches the dst order
                    nc.sync.dma_start(
                        _ap(zl, 0, [16 * KSHP, OUT_CH], [KSHP, GL], [1, D]),
                        s1h[0:1, :])

                    nc.gpsimd.collective_compute(
                        "AllToAll", AL.bypass,
                        replica_groups=[list(range(NCORES))],
                        ins=[zl[:].opt()], outs=[zex[:].opt()])

                # ---- z0 = z @ w0 partial over this core's K shard ----
                with tc.tile_pool(name="zt", bufs=1) as ztp, \
                     tc.tile_pool(name="ptr", bufs=2, space="PSUM") as ptrp, \
                     tc.tile_pool(name="pz0", bufs=1, space="PSUM") as pz0p, \
                     tc.tile_pool(name="ptail", bufs=1, space="PSUM") as ptl:
                    # one fully-contiguous batch-major gather (7.4 KB runs),
                    # then PE-transpose each [64, 128] block to [128 K, 64 b]
                    zb = ztp.tile([B, KSHP], F16, tag="zb", name="zb")
                    nc.sync.dma_start(zb[:], zex[:])
                    zT = [ztp.tile([128, 14 * B], F16, tag="zta", name="zta"),
                          ztp.tile([128, 14 * B], F16, tag="ztb", name="ztb"),
                          ztp.tile([128, B], F16, tag="ztc", name="ztc")]

                    def zt_chunk(i):
                        if i < 28:
                            return zT[i // 14][:, (i % 14) * B:
                                               (i % 14 + 1) * B]
                        return zT[2][:]

                    # all transposes first (HAM ignores transpose-mode, so
                    # interleaving would keep the PE cold), then a dense
                    # matmul burst that warms up and stays warm
                    for i in range(NCH):
                        ptr = ptrp.tile([128, B], F16, tag="ptr")
                        nc.tensor.transpose(ptr[:],
                                            zb[:, i * 128:(i + 1) * 128],
                                            idh_sb[:])
                        if i % 2 == 0:
                            nc.vector.tensor_copy(zt_chunk(i), ptr[:])
                        else:
                            nc.scalar.activation(zt_chunk(i), ptr[:], AF.Copy)
                    z0p = pz0p.tile([B, H0], F32, tag="z0p")
                    for i in range(NCH):
                        if i < 28:
                            rhs = w0t[i // 14][:, (i % 14) * H0:
                                               (i % 14 + 1) * H0]
                        else:
                            rhs = w0t[2][:]
                        nc.tensor.matmul(z0p[:], zt_chunk(i), rhs,
                                         start=(i == 0), stop=(i == NCH - 1))
                    z0sb = cpool.tile([B, H0], F32, tag="z0sb", name="z0sb")
                    nc.vector.tensor_copy(z0sb[:], z0p[:])
                    nc.sync.dma_start(cin[:], z0sb[:])
                    nc.gpsimd.collective_compute(
                        "ReduceScatter", AL.add,
                        replica_groups=[list(range(NCORES))],
                        ins=[cin[:].opt()], outs=[cout[:].opt()])

                    # preload ACT tables during the RS window (dead time,
                    # and after the last AF.Copy use so nothing re-evicts)
                    dum = ppool.tile([1, 4], F32, tag="dum")
                    nc.scalar.activation(dum[0:1, 0:1], tl_sb[0:1, 0:1],
                                         AF.Sigmoid)
                    nc.scalar.activation(dum[0:1, 1:2], tl_sb[0:1, 0:1],
                                         AF.Exp)
                    nc.scalar.activation(dum[0:1, 2:3], onef_sb[0:1, 0:1],
                                         AF.Ln)

                    # ---- tail: sigmoid(z0) -> w1 -> sigmoid -> w2 ----
                    z1row = cpool.tile([BL, H0], F32, tag="z1row")
                    nc.sync.dma_start(z1row[:], cout[:])
                    pz1 = ptl.tile([128, 4 * BL], F32, tag="pz1")
                    z1t = cpool.tile([128, 4 * BL], F16, tag="z1t")
                    for j in range(4):
                        nc.tensor.transpose(pz1[:, j * BL:(j + 1) * BL],
                                            z1row[:, j * 128:(j + 1) * 128],
                                            idn_sb[0:BL, 0:BL])
                        nc.scalar.activation(z1t[:, j * BL:(j + 1) * BL],
                                             pz1[:, j * BL:(j + 1) * BL],
                                             AF.Sigmoid, bias=b0_sb[:, j:j + 1])
                    pz2 = ptl.tile([128, 2 * BL], F32, tag="pz2")
                    z2t = cpool.tile([128, 2 * BL], F16, tag="z2t")
                    for m in range(2):
                        for kj in range(4):
                            nc.tensor.matmul(
                                pz2[:, m * BL:(m + 1) * BL],
                                w1_sb[kj][:, m * 128:(m + 1) * 128],
                                z1t[:, kj * BL:(kj + 1) * BL],
                                start=(kj == 0), stop=(kj == 3))
                        nc.scalar.activation(z2t[:, m * BL:(m + 1) * BL],
                                             pz2[:, m * BL:(m + 1) * BL],
                                             AF.Sigmoid, bias=b1_sb[:, m:m + 1])
                    pz3 = ptl.tile([BL, NCLS], F32, tag="pz3")
                    for m in range(2):
                        nc.tensor.matmul(pz3[:], z2t[:, m * BL:(m + 1) * BL],
                                         w2_sb[m][:], start=(m == 0),
                                         stop=(m == 1))
                    z3 = cpool.tile([BL, NCLS], F32, tag="z3")
                    nc.vector.tensor_tensor(z3[:], pz3[:], b2_sb[:], AL.add)
                    mx = cpool.tile([BL, 1], F32, tag="mx")
                    nc.vector.tensor_reduce(mx[:], z3[:], mybir.AxisListType.X,
                                            AL.max)
                    tm = cpool.tile([BL, NCLS], F32, tag="tm")
                    nc.vector.tensor_scalar(tm[:], z3[:], mx[:, 0:1], None,
                                            AL.subtract)
                    ex = cpool.tile([BL, NCLS], F32, tag="ex")
                    se = cpool.tile([BL, 1], F32, tag="se")
                    nc.scalar.activation(ex[:], tm[:], AF.Exp, accum_out=se[:])
                    ls = cpool.tile([BL, 1], F32, tag="ls")
                    nc.scalar.activation(ls[:], se[:], AF.Ln)
                    osb = cpool.tile([BL, NCLS], F32, tag="osb")
                    nc.vector.tensor_scalar(osb[:], tm[:], ls[:, 0:1], None,
                                            AL.subtract)
                    nc.sync.dma_start(out[:], osb[:])
    return nc


_CACHE = {}


def kernel(x, conv_w, conv_b, w0, b0, w1, b1, w2, b2):
    x = np.ascontiguousarray(np.asarray(x, np.float32))
    conv_w = np.asarray(conv_w, np.float32)
    conv_b = np.asarray(conv_b, np.float32)
    w0 = np.asarray(w0, np.float32)
    w1 = np.ascontiguousarray(np.asarray(w1, np.float32))
    w2 = np.ascontiguousarray(np.asarray(w2, np.float32))
    b0 = np.asarray(b0, np.float32)
    b1 = np.asarray(b1, np.float32)
    b2 = np.asarray(b2, np.float32)

    if "nc" not in _CACHE:
        _CACHE["nc"] = _build()
        _CACHE["perm"] = _w0_perm()
        # S3/S4 features arrive /4; S1/S2 natural scale
        sc = np.ones(SIGC, np.float32)
        sc[90:] = FSCALE
        _CACHE["wscale"] = np.tile(sc, OUT_CH)
    nc = _CACHE["nc"]
    w0p = (w0[_CACHE["perm"], :]
           * _CACHE["wscale"][:, None]).astype(np.float16)
    w0pp = np.zeros((NCORES, KSHP, H0), np.float16)
    w0pp[:, :KSH, :] = w0p.reshape(NCORES, KSH, H0)

    shared = {
        "cwr": np.ascontiguousarray(
            np.broadcast_to(conv_w.reshape(1, 16), (128, 16))),
        "cbr": np.ascontiguousarray(
            np.broadcast_to(conv_b.reshape(1, OUT_CH), (128, OUT_CH))),
        "tlin": np.linspace(0.0, 1.0, T, dtype=np.float32).reshape(128, 1),
        "ltri": np.ascontiguousarray(
            np.triu(np.ones((128, 128), np.float32), 1)).astype(np.float16),
        "onec": np.ones((128, 1), np.float16),
        "idh": np.ascontiguousarray(np.eye(64, dtype=np.float16)),
        "onef": np.ones((128, 1), np.float32),
        "idn": np.ascontiguousarray(np.eye(128, BL, dtype=np.float32)),
        "w1s": w1.astype(np.float16), "w2s": w2.astype(np.float16),
        "b0c": b0.reshape(H0, 1), "b1c": b1.reshape(H1, 1),
        "b2r": np.ascontiguousarray(np.broadcast_to(b2.reshape(1, NCLS),
                                                    (BL, NCLS))),
    }
    in_maps = []
    for c in range(NCORES):
        m = dict(shared)
        m["xs"] = np.ascontiguousarray(
            x[c * BL:(c + 1) * BL, 0].transpose(1, 0, 2))
        m["w0s"] = np.ascontiguousarray(w0pp[c])
        in_maps.append(m)

    _CACHE["in_maps"] = in_maps
    res = run_bass_kernel_spmd(nc, in_maps, core_ids=list(range(NCORES)))
    return np.concatenate([res.results[c]["out"] for c in range(NCORES)],
                          axis=0)
